# revision 33
# baseline (speedup 1.0000x reference)
"""Trainium2 Bass kernel for nn_ApproxSymmetricNet (gnn_message_passing).

8 NeuronCores, hybrid sharding: core c = (g=c//4 batch-half, p=c%4
plaquette-quarter).

chi is SITE-sharded by 8 (2048 sites x all 128 batches per core; 9 gather
descriptors per site). wilson/omega are (batch-half x plaquette-quarter)
sharded: rows hold 64 batches -> 512 fp16 = 1KB elements, so gather
descriptor counts drop to 4x4096 and 5x4096 per core (the SWDGE gather
ucode costs ~571ns + 7.86ns/descriptor of serial Pool time, independent
of element size -- the kernel's governing cost).

Dataflow: chi (gather x rows -> PE conv -> complex tanh -> PE transpose to
site-major fp16 rows [b64, ri, i]) -> dup-staged AllToAll (each dest quad
gets its batch-half) -> wilson (gather h1 rows, DVE products) -> quad
AllGather -> omega (gather h2 rows -> DRAM scratch -> HWDGE DMA-transpose
puts comps on partitions (PE transpose of gathered volume would be
PE-bound; transpose-mode dma_gather crashes this runtime) -> 5 f16
block-diag matmuls (K=128=(b_lo,ri,i) -> M=(ri_o,b_lo,o)) -> complex tanh
(reciprocal on ACT) -> free-dim reduce). Host sums channels + quarters.

tanh(x+iy) = (2T + i*(1-T^2)*sin2y) / D, D = 2*(1 - (1-T^2)*sin^2(y)).
"""
import numpy as np

import concourse.bacc as bacc
import concourse.mybir as mybir
import concourse.tile as tile
from concourse.bass_utils import run_bass_kernel_spmd

AFT = mybir.ActivationFunctionType
ALU = mybir.AluOpType
F32 = mybir.dt.float32
F16 = mybir.dt.float16
I16 = mybir.dt.int16

B, N_SITES, N_PLAQ = 128, 16384, 16384
K_CHI, P_SZ, K_OMG = 9, 4, 5
C_CHI, C_OMG = 4, 4
WILSON_RESCALE = 10 ** 1.5
NCORES = 8
NG, NP = 2, 4               # batch groups x plaquette quarters
BG = B // NG                # 64 batches per group
S_LOC = N_SITES // NCORES   # 2048 sites per core (chi)
P_LOC = N_PLAQ // NP        # 4096 plaquettes per core (wilson/omega)

# chi chunking (site-sharded)
DN = 14                     # sites per partition-group (14*9=126 partitions)
CHI_COLS = 4
CHI_SITES = DN * CHI_COLS   # 56 sites per chunk
CHI_NCH = (S_LOC + CHI_SITES - 1) // CHI_SITES      # 37

NQ = 1                      # single a2a
SQ = S_LOC
WIL_PC = 256                # wilson plaquettes per chunk (4*256=1024 idx)
WIL_NCH = P_LOC // WIL_PC   # 16
OMG_PC = 1024               # omega plaquettes per chunk (5 calls of 1024)
OMG_NCH = P_LOC // OMG_PC   # 4
AG_PC = 1024                # h2 AllGather granularity (4 chunks)
DEBUG_DUMP = False


def _wrap_idx16(flat):
    n = len(flat)
    a = flat.reshape(n // 16, 16).T
    return np.tile(a, (8, 1)).astype(np.int16)


def _h1row(n):
    # site n -> row in d_recv [8 src][2048]: identity
    return n


def _h2row(m):
    # plaquette m -> row in d_h2g [4j][4rank][1024]
    return (m % P_LOC) // AG_PC * (NP * AG_PC) + (m // P_LOC) * AG_PC \
        + m % AG_PC


def build_host_tables(chi_kernel_idx, plaquette_idx, omega_kernel_idx,
                      chi_w, omega_w):
    # ---- per-core chi gather tables (site shard c) ----
    ci = np.concatenate(
        [chi_kernel_idx, np.full((CHI_SITES, K_CHI), N_SITES, np.int64)])
    chi_gidx_cores = []
    j = np.arange(CHI_NCH * CHI_COLS * 128)
    col = j // 128
    pp = j % 128
    dn = pp // K_CHI
    k = pp % K_CHI
    nl = (j // (CHI_COLS * 128)) * CHI_SITES + col % CHI_COLS * DN \
        + np.minimum(dn, DN - 1)
    for cc in range(NCORES):
        n = np.where(nl < S_LOC, cc * S_LOC + nl, N_SITES)
        flat = ci[np.minimum(n, N_SITES), k]
        flat[pp >= DN * K_CHI] = 0
        chi_gidx_cores.append(_wrap_idx16(flat))

    # ---- per-quarter wilson gather tables ----
    wil_gidx_p = []
    for p in range(NP):
        flat = np.zeros(WIL_NCH * 4 * WIL_PC, np.int64)
        for ch in range(WIL_NCH):
            jj = np.arange(4 * WIL_PC)
            kk = jj // WIL_PC
            cc_ = (jj % WIL_PC) // 128
            qq = jj % 128
            m = p * P_LOC + ch * WIL_PC + cc_ * 128 + qq
            flat[ch * 4 * WIL_PC + jj] = _h1row(plaquette_idx[m, kk])
        wil_gidx_p.append(_wrap_idx16(flat))

    # ---- per-quarter omega gather tables ----
    omg_gidx_p = []
    for p in range(NP):
        flat = np.zeros(OMG_NCH * K_OMG * OMG_PC, np.int64)
        pos = 0
        for ch in range(OMG_NCH):
            for k in range(K_OMG):
                m = p * P_LOC + ch * OMG_PC + np.arange(OMG_PC)
                flat[pos:pos + OMG_PC] = _h2row(omega_kernel_idx[m, k])
                pos += OMG_PC
        omg_gidx_p.append(_wrap_idx16(flat))

    # ---- chi weight lhsT [128,128]: row (dn*9+k) -> col (ri*64+dn*4+i) ----
    wchi = np.zeros((128, 128), np.float32)
    for dn_ in range(DN):
        for k in range(K_CHI):
            for i in range(C_CHI):
                wchi[dn_ * K_CHI + k, 0 * 64 + dn_ * 4 + i] = \
                    chi_w[i, 0, k].real
                wchi[dn_ * K_CHI + k, 1 * 64 + dn_ * 4 + i] = \
                    chi_w[i, 0, k].imag

    # ---- omega weights [128, 5*128] f16:
    # row (b_lo*8 + ri*4 + i) -> col (ri_o*64 + b_lo*4 + o) ----
    woms = []
    for k in range(K_OMG):
        w = np.zeros((128, 128), np.float32)
        for bl in range(16):
            for i in range(C_CHI):
                for o in range(C_OMG):
                    wr = omega_w[o, i, k].real
                    wi = omega_w[o, i, k].imag
                    w[bl * 8 + 0 * 4 + i, 0 * 64 + bl * 4 + o] = wr
                    w[bl * 8 + 1 * 4 + i, 0 * 64 + bl * 4 + o] = -wi
                    w[bl * 8 + 0 * 4 + i, 1 * 64 + bl * 4 + o] = wi
                    w[bl * 8 + 1 * 4 + i, 1 * 64 + bl * 4 + o] = wr
        woms.append(w)
    womg = np.concatenate(woms, axis=1).astype(np.float16)
    return chi_gidx_cores, wil_gidx_p, omg_gidx_p, wchi, womg


def emit_ctanh(nc, pool, pslist, out_re, out_im, tagpfx):
    """Complex tanh of stacked psum (re, im) pairs.

    pslist: list of (psum_x[H,F], psum_y[H,F]); out_re/out_im [sum(H), F]
    SBUF APs. ACT: Tanh then Sins (batched to limit table reloads);
    DVE: 7 f16 ops + reciprocal.
    """
    H = pslist[0][0].shape[0]
    P = out_re.shape[0]
    F = out_re.free_size()

    def t(nm):
        return pool.tile([P, F], F16, name=f"{tagpfx}_{nm}", tag=f"ct_{nm}",
                         bufs=2)
    T_, s_, c_, t2, q_, u_, d_, r_ = (t(x) for x in
                                      ("T", "s", "c", "t2", "q", "u", "d",
                                       "r"))
    for ui, (sx, sy) in enumerate(pslist):
        sl = slice(ui * 64, ui * 64 + H)
        nc.scalar.activation(T_[sl, :], sx, AFT.Tanh)
    for ui, (sx, sy) in enumerate(pslist):
        sl = slice(ui * 64, ui * 64 + H)
        nc.scalar.activation(s_[sl, :], sy, AFT.Sin, scale=2.0)
        nc.scalar.activation(c_[sl, :], sy, AFT.Sin)
    nc.vector.tensor_mul(t2[:], T_[:], T_[:])
    nc.vector.tensor_scalar(out=q_[:], in0=t2[:], scalar1=-1.0, scalar2=1.0,
                            op0=ALU.mult, op1=ALU.add)
    nc.vector.tensor_mul(u_[:], c_[:], c_[:])
    nc.vector.tensor_mul(u_[:], u_[:], q_[:])
    nc.vector.tensor_scalar(out=d_[:], in0=u_[:], scalar1=-2.0, scalar2=2.0,
                            op0=ALU.mult, op1=ALU.add)
    nc.vector.reciprocal(r_[:], d_[:])
    nc.vector.scalar_tensor_tensor(out=out_re, in0=T_[:], scalar=2.0,
                                   in1=r_[:], op0=ALU.mult, op1=ALU.mult)
    nc.vector.tensor_mul(s_[:], s_[:], q_[:])
    nc.vector.tensor_mul(out_im, s_[:], r_[:])


def emit_taylor_ctanh(nc, pool, za, zb, out_re, out_im):
    """tanh(w) ~= w - w^3/3 = w*(1 - w^2/3) for |w| <~ 0.2 (omega's z is
    <~ 0.05 for this model scale; validated host-side). DVE-only: frees the
    ACT engine and psum banks quickly. za/zb: psum [128,512] with re in
    partitions [0:64], im in [64:128]; stacked as two units."""
    def t(nm):
        return pool.tile([128, 512], F16, name=f"tl_{nm}", tag=f"tl_{nm}",
                         bufs=2)
    zx, zy, x2, y2, q2, p_, pr, t1, v1, v3, v4 = (
        t(x) for x in ("zx", "zy", "x2", "y2", "q2", "p", "pr", "t1", "v1",
                       "v3", "v4"))
    nc.vector.tensor_copy(out=zx[0:64, :], in_=za[0:64, :])
    nc.vector.tensor_copy(out=zx[64:128, :], in_=zb[0:64, :])
    nc.vector.tensor_copy(out=zy[0:64, :], in_=za[64:128, :])
    nc.vector.tensor_copy(out=zy[64:128, :], in_=zb[64:128, :])
    nc.vector.tensor_mul(x2[:], zx[:], zx[:])
    nc.vector.tensor_mul(y2[:], zy[:], zy[:])
    nc.vector.tensor_mul(q2[:], zx[:], zy[:])
    nc.vector.scalar_tensor_tensor(out=p_[:], in0=y2[:], scalar=-1.0,
                                   in1=x2[:], op0=ALU.mult, op1=ALU.add)
    nc.vector.tensor_scalar(out=pr[:], in0=p_[:], scalar1=-1.0 / 3.0,
                            scalar2=1.0, op0=ALU.mult, op1=ALU.add)
    nc.vector.tensor_mul(t1[:], zx[:], pr[:])
    nc.vector.tensor_mul(v1[:], zy[:], q2[:])
    nc.vector.scalar_tensor_tensor(out=out_re, in0=v1[:], scalar=2.0 / 3.0,
                                   in1=t1[:], op0=ALU.mult, op1=ALU.add)
    nc.vector.tensor_mul(v3[:], zx[:], q2[:])
    nc.vector.tensor_mul(v4[:], zy[:], pr[:])
    nc.vector.scalar_tensor_tensor(out=out_im, in0=v3[:], scalar=-2.0 / 3.0,
                                   in1=v4[:], op0=ALU.mult, op1=ALU.add)


def build_kernel():
    nc = bacc.Bacc("TRN2", target_bir_lowering=False, debug=True)

    d_xf = nc.dram_tensor("xf", [N_SITES + 1, 128], F32, kind="ExternalInput")
    d_cgi = nc.dram_tensor("cgi", [128, CHI_NCH * CHI_COLS * 8], I16,
                           kind="ExternalInput")
    d_wgi = nc.dram_tensor("wgi", [128, WIL_NCH * 64], I16,
                           kind="ExternalInput")
    d_ogi = nc.dram_tensor("ogi", [128, OMG_NCH * K_OMG * (OMG_PC // 16)], I16,
                           kind="ExternalInput")
    d_wchi = nc.dram_tensor("wchi", [128, 128], F32, kind="ExternalInput")
    d_womg = nc.dram_tensor("womg", [128, K_OMG * 128], F16,
                            kind="ExternalInput")
    # a2a staging: per site-half, 8 dest blocks of [1024 sites, 512 f16]
    d_stq = [nc.dram_tensor(f"stq{q}", [NCORES, SQ, 512], F16)
             for q in range(NQ)]
    # h1 for my batch group: rows q*8192 + src*1024 + loc
    d_recv = nc.dram_tensor("recv", [NQ, NCORES, SQ, 512], F16)
    # wilson output shard, j-major quarters of my 4096 plaquettes
    d_h2q = [nc.dram_tensor(f"h2q{j}", [AG_PC, 512], F16)
             for j in range(4)]
    # h2 full for my batch group: rows j*4096 + rank*1024 + loc
    d_h2g = nc.dram_tensor("h2g", [4, NP, AG_PC, 512], F16)
    if DEBUG_DUMP:
        d_recv_d = nc.dram_tensor("recv_d", [NQ * NCORES * SQ, 512], F16,
                                  kind="ExternalOutput")
        d_h2g_d = nc.dram_tensor("h2g_d", [4 * NP * AG_PC, 512], F16,
                                 kind="ExternalOutput")
        d_gt_d = nc.dram_tensor("gt_d", [128, OMG_PC], F16,
                                kind="ExternalOutput")
        d_zc_d = nc.dram_tensor("zc_d", [128, 512], F32,
                                kind="ExternalOutput")
        d_tr_d = nc.dram_tensor("tr_d", [128, 1024], F32,
                                kind="ExternalOutput")
    # omega gather scratch (double-buffered over chunks)
    d_gsc = nc.dram_tensor("gsc", [OMG_NCH, K_OMG, OMG_PC, 512], F16)
    d_out = nc.dram_tensor("out", [64, 8], F32, kind="ExternalOutput")

    with tile.TileContext(nc) as tc, \
         nc.allow_low_precision(reason="fp16 rows/tanh; tol 2e-2"):
        with tc.tile_pool(name="pidx", bufs=1) as pidx, \
             tc.tile_pool(name="pwork", bufs=1) as pool, \
             tc.tile_pool(name="ppsum", bufs=1, space="PSUM") as ppsum:
            t_cgi = pidx.tile([128, CHI_NCH * CHI_COLS * 8], I16,
                              name="t_cgi")
            t_wgi = pidx.tile([128, WIL_NCH * 64], I16, name="t_wgi")
            t_ogi = pidx.tile([128, OMG_NCH * K_OMG * (OMG_PC // 16)], I16,
                              name="t_ogi")
            t_wchi = pidx.tile([128, 128], F32, name="t_wchi")
            t_womg = pidx.tile([128, K_OMG * 128], F16, name="t_womg")
            nc.sync.dma_start(t_cgi[:], d_cgi[:])
            nc.sync.dma_start(t_wgi[:], d_wgi[:])
            nc.sync.dma_start(t_ogi[:], d_ogi[:])
            nc.sync.dma_start(t_wchi[:], d_wchi[:])
            nc.sync.dma_start(t_womg[:], d_womg[:])

            # =========== chi (site-sharded, all 128 batches) ===========
            # h1 row content is [i(4), ri(2), b(64)]: the stage write from
            # ctanh's [(dn,i) parts, (col,b)] layout is then a 3-dim AP with
            # 128B runs: merged (dn,i) stride 128-els, col stride 7168, b 1.
            q_written = [0] * NQ

            def stage_write(h1re, h1im, npair, pr):
                nonlocal dma_flip
                for u in range(npair):
                    s0 = (pr + u) * CHI_SITES
                    # rectangles (quarter, col0, ncol, dn0, ndn) not crossing
                    # a quarter boundary nor S_LOC
                    pieces = []
                    for colq in range(CHI_COLS):
                        base = s0 + colq * DN
                        ns = min(DN, max(0, S_LOC - base))
                        st = 0
                        while st < ns:
                            s = base + st
                            qh = s // SQ
                            n_ = min(ns - st, (qh + 1) * SQ - s)
                            pieces.append((qh, colq, st, n_))
                            st += n_
                    rects = []
                    ii = 0
                    while ii < len(pieces):
                        qh, colq, st, n_ = pieces[ii]
                        if st == 0 and n_ == DN:
                            jj = ii
                            while (jj + 1 < len(pieces)
                                   and pieces[jj + 1][:1] == (qh,)
                                   and pieces[jj + 1][2] == 0
                                   and pieces[jj + 1][3] == DN
                                   and pieces[jj + 1][1] == pieces[jj][1] + 1):
                                jj += 1
                            rects.append((qh, colq, pieces[jj][1] - colq + 1,
                                          0, DN))
                            ii = jj + 1
                        else:
                            rects.append((qh, colq, 1, st, n_))
                            ii += 1
                    for (qh, c0, ncol, dn0, ndn) in rects:
                        # group index m = local_site*4 + i; multi-col rects
                        # are always full cols (dn0=0, ndn=DN)
                        m0 = (s0 - qh * SQ + c0 * DN + dn0) * 4
                        for ri, tl in ((0, h1re), (1, h1im)):
                            for gg in range(NG):
                                M = d_stq[qh][gg * NP].rearrange(
                                    "s (m x) -> (s m) x", m=4)
                                if ncol == 1:
                                    dst = M[m0:m0 + 4 * ndn,
                                            ri * 64:(ri + 1) * 64]
                                else:
                                    dst = M[m0:m0 + ncol * 4 * DN, :] \
                                        .rearrange("(c m) x -> m c x",
                                                   c=ncol)[
                                        :, :, ri * 64:(ri + 1) * 64]
                                src = tl[u * 64 + dn0 * 4:
                                         u * 64 + (dn0 + ndn) * 4,
                                         c0:c0 + ncol, gg * 64:(gg + 1) * 64]
                                if ncol == 1:
                                    src = src.rearrange("p c b -> p (c b)")
                                eng = nc.sync if dma_flip else nc.scalar
                                dma_flip = not dma_flip
                                eng.dma_start(dst, src)
                        q_written[qh] += ncol * ndn

            dups_done = 0
            dma_flip = True
            for pr in range(0, CHI_NCH, 2):
                npair = min(2, CHI_NCH - pr)
                pss = []
                for u in range(npair):
                    ch = pr + u
                    gch = pool.tile([128, CHI_COLS, 128], F32, name="gchi",
                                    tag="gchi", bufs=3)
                    nc.gpsimd.dma_gather(
                        gch[:], d_xf[:],
                        t_cgi[:, ch * CHI_COLS * 8:(ch + 1) * CHI_COLS * 8],
                        CHI_COLS * 128, CHI_COLS * 128, 128, elem_step=128)
                    pch = ppsum.tile([128, 512], F32, name="pchi", tag="pchi",
                                     bufs=2)
                    nc.tensor.matmul(
                        pch[:], lhsT=t_wchi[:],
                        rhs=gch[:].rearrange("p a b -> p (a b)"),
                        start=True, stop=True)
                    pss.append(pch)
                P = 64 * (npair - 1) + 56
                h1re = pool.tile([120, CHI_COLS, 128], F16, name="h1re",
                                 tag="h1re", bufs=3)
                h1im = pool.tile([120, CHI_COLS, 128], F16, name="h1im",
                                 tag="h1im", bufs=3)
                emit_ctanh(
                    nc, pool,
                    [(p_[0:56, :], p_[64:120, :]) for p_ in pss],
                    h1re[0:P, :, :].rearrange("p a b -> p (a b)"),
                    h1im[0:P, :, :].rearrange("p a b -> p (a b)"), "ctchi")
                stage_write(h1re, h1im, npair, pr)
                # pipeline the dup copies in row-halves behind chi; fire
                # the single a2a once everything is staged
                while dups_done < 2 and q_written[0] >= (dups_done + 1) * 1024:
                    r0, r1 = dups_done * 1024, (dups_done + 1) * 1024
                    for gg in range(NG):
                        for pp_ in range(1, NP):
                            eng = nc.sync if dma_flip else nc.scalar
                            dma_flip = not dma_flip
                            eng.dma_start(
                                d_stq[0][gg * NP + pp_, r0:r1, :],
                                d_stq[0][gg * NP, r0:r1, :])
                    dups_done += 1
            nc.gpsimd.collective_compute(
                "AllToAll", ALU.bypass,
                replica_groups=[list(range(NCORES))],
                ins=[d_stq[0][:]], outs=[d_recv[0]])

            # =========== wilson (quarter p, my 64 batches) ===========
            recv_flat = d_recv[:].rearrange("a b c f -> (a b c) f")
            for ch in range(WIL_NCH):
                gw = pool.tile([128, 4, 2, 512], F16, name="gwil", tag="gwil",
                               bufs=3)
                nc.gpsimd.dma_gather(
                    gw[:].rearrange("p a b f -> p (a b) f"), recv_flat,
                    t_wgi[:, ch * 64:(ch + 1) * 64], 4 * WIL_PC, 4 * WIL_PC,
                    512, elem_step=512)
                m1 = pool.tile([128, 2, 512], F16, name="wm1", tag="wm1",
                               bufs=2)
                m2 = pool.tile([128, 2, 512], F16, name="wm2", tag="wm2",
                               bufs=2)
                h2t = pool.tile([128, 2, 512], F16, name="h2t", tag="h2t",
                                bufs=2)
                nc.vector.tensor_mul(m1[:], gw[:, 0, :, :], gw[:, 1, :, :])
                nc.vector.tensor_mul(m2[:], gw[:, 2, :, :], gw[:, 3, :, :])
                # content reorder [i,ri,b] -> [b,ri,i] for omega's
                # K-slices; DVE APs are limited to 3D, so split by (cc, ri)
                for cc in range(2):
                    for ri in range(2):
                        dstv = h2t[:, cc, :].rearrange(
                            "p (b r i) -> r p b i", b=64, r=2, i=4)[ri]
                        nc.vector.scalar_tensor_tensor(
                            out=dstv,
                            in0=m1[:, cc, :].rearrange(
                                "p (i r b) -> r p b i", i=4, r=2, b=64)[ri],
                            scalar=float(WILSON_RESCALE),
                            in1=m2[:, cc, :].rearrange(
                                "p (i r b) -> r p b i", i=4, r=2, b=64)[ri],
                            op0=ALU.mult, op1=ALU.mult)
                jq, r0 = ch // 4, (ch % 4) * WIL_PC
                nc.sync.dma_start(
                    d_h2q[jq][r0:r0 + WIL_PC, :]
                    .rearrange("(c q) f -> q c f", c=2), h2t[:])
                if ch % 4 == 3:
                    nc.gpsimd.collective_compute(
                        "AllGather", ALU.bypass,
                        replica_groups=[[0, 1, 2, 3], [4, 5, 6, 7]],
                        ins=[d_h2q[jq][:]], outs=[d_h2g[jq]])

            # =========== omega (quarter p, my 64 batches) ===========
            h2g_flat = d_h2g[:].rearrange("a b c f -> (a b c) f")
            acc_re = pool.tile([64, 4], F32, name="acc_re")
            acc_im = pool.tile([64, 4], F32, name="acc_im")
            nc.vector.memset(acc_re[:], 0.0)
            nc.vector.memset(acc_im[:], 0.0)

            def omega_tail(zcs, grp):
                # zcs[j]: bh = 2*grp + j//2, half = j%2; pair (pr_, pr_+2)
                # stacks bh 2*grp (rows 0:64) with bh 2*grp+1 (rows 64:128)
                for pr_ in range(2):
                    za, zb = zcs[pr_], zcs[pr_ + 2]
                    tr = pool.tile([128, 512], F32, name="otr", tag="otr",
                                   bufs=2)
                    ti = pool.tile([128, 512], F32, name="oti", tag="oti",
                                   bufs=2)
                    emit_taylor_ctanh(nc, pool, za, zb, tr[:], ti[:])
                    rr = pool.tile([128, 1], F32, name="orr", tag="orr",
                                   bufs=2)
                    rim = pool.tile([128, 1], F32, name="ori", tag="ori",
                                    bufs=2)
                    nc.vector.tensor_reduce(
                        out=rr[:], in_=tr[:], axis=mybir.AxisListType.X,
                        op=ALU.add)
                    nc.vector.tensor_reduce(
                        out=rim[:], in_=ti[:], axis=mybir.AxisListType.X,
                        op=ALU.add)
                    bha, bhb = 2 * grp, 2 * grp + 1
                    nc.vector.tensor_add(
                        acc_re[:, bha:bha + 1], acc_re[:, bha:bha + 1],
                        rr[0:64, :])
                    nc.vector.tensor_add(
                        acc_im[:, bha:bha + 1], acc_im[:, bha:bha + 1],
                        rim[0:64, :])
                    r2 = pool.tile([64, 1], F32, name="or2", tag="or2",
                                   bufs=2)
                    i2 = pool.tile([64, 1], F32, name="oi2", tag="oi2",
                                   bufs=2)
                    nc.vector.tensor_copy(out=r2[:], in_=rr[64:128, :])
                    nc.vector.tensor_copy(out=i2[:], in_=rim[64:128, :])
                    nc.vector.tensor_add(
                        acc_re[:, bhb:bhb + 1], acc_re[:, bhb:bhb + 1],
                        r2[:])
                    nc.vector.tensor_add(
                        acc_im[:, bhb:bhb + 1], acc_im[:, bhb:bhb + 1],
                        i2[:])

            for ch in range(OMG_NCH):
                buf = ch
                for k in range(K_OMG):
                    gk = pool.tile([128, OMG_PC // 128, 512], F16,
                                   name="gomg", tag="gomg", bufs=3)
                    off = (ch * K_OMG + k) * (OMG_PC // 16)
                    nc.gpsimd.dma_gather(
                        gk[:], h2g_flat, t_ogi[:, off:off + OMG_PC // 16],
                        OMG_PC, OMG_PC, 512, elem_step=512)
                    nc.scalar.dma_start(
                        d_gsc[buf, k].rearrange("(c q) f -> q c f",
                                                c=OMG_PC // 128),
                        gk[:])
                for grp in range(2):
                    gTs = []
                    for k in range(K_OMG):
                        gT = pool.tile([128, 2, OMG_PC], F16, name="gT",
                                       tag="gT", bufs=6)
                        for bi in range(2):
                            bh = 2 * grp + bi
                            nc.sync.dma_start_transpose(
                                gT[:, bi, :],
                                d_gsc[buf, k][:, 128 * bh:128 * (bh + 1)])
                        gTs.append(gT)
                    for nh in range(OMG_PC // 1024):
                        zcs = [ppsum.tile([128, 512], F32, name=f"zc{jj}",
                                          tag=f"zc{jj}", bufs=1)
                               for jj in range(4)]
                        for k in range(K_OMG):
                            for bi in range(2):
                                for hh in range(2):
                                    nc.tensor.matmul(
                                        zcs[bi * 2 + hh][:],
                                        lhsT=t_womg[:,
                                                    k * 128:(k + 1) * 128],
                                        rhs=gTs[k][:, bi,
                                                   nh * 1024 + 512 * hh:
                                                   nh * 1024 + 512 * (hh + 1)
                                                   ],
                                        start=(k == 0),
                                        stop=(k == K_OMG - 1))
                        omega_tail(zcs, grp)

            if DEBUG_DUMP:
                nc.sync.dma_start(
                    d_recv_d[:],
                    d_recv[:].rearrange("a b c f -> (a b c) f"))
                nc.sync.dma_start(
                    d_h2g_d[:],
                    d_h2g[:].rearrange("a b c f -> (a b c) f"))
            out_t = pool.tile([64, 8], F32, name="out_t")
            nc.vector.tensor_copy(out=out_t[:, 0:4], in_=acc_re[:])
            nc.vector.tensor_copy(out=out_t[:, 4:8], in_=acc_im[:])
            nc.sync.dma_start(d_out[:], out_t[:])
    nc.compile()
    return nc


_NC_CACHE = None


def kernel(x, chi_kernel_idx, chi_kernel_mask, plaquette_idx, plaquette_mask,
           omega_kernel_idx, omega_kernel_mask, chi_w, chi_b, omega_w,
           omega_b, _want_trace=False):
    global _NC_CACHE
    x = np.asarray(x, np.float32)
    chi_kernel_idx = np.asarray(chi_kernel_idx).astype(np.int64)
    plaquette_idx = np.asarray(plaquette_idx).astype(np.int64)
    omega_kernel_idx = np.asarray(omega_kernel_idx).astype(np.int64)
    chi_w = np.asarray(chi_w)
    omega_w = np.asarray(omega_w)

    chi_gidx_cores, wil_gidx_p, omg_gidx_p, wchi, womg = build_host_tables(
        chi_kernel_idx, plaquette_idx, omega_kernel_idx, chi_w, omega_w)

    # omega uses a cubic Taylor tanh; verify |z_omega| is in-range
    _h1 = np.tanh(np.einsum(
        "bnk,ik->bni", x.astype(np.complex64)[:, chi_kernel_idx],
        chi_w[:, 0, :]))
    _gp = _h1[:, plaquette_idx, :]
    _h2 = (WILSON_RESCALE * _gp.real.prod(axis=2)
           + 1j * WILSON_RESCALE * _gp.imag.prod(axis=2))
    _z = np.einsum("bnki,oik->bno", _h2[:, omega_kernel_idx, :], omega_w)
    _zmax = max(np.abs(_z.real).max(), np.abs(_z.imag).max())
    assert _zmax < 0.25, f"omega pre-tanh out of Taylor range: {_zmax}"

    if _NC_CACHE is None:
        _NC_CACHE = build_kernel()
    nc = _NC_CACHE

    xf = np.zeros((N_SITES + 1, 128), np.float32)
    xf[:N_SITES] = x.T
    in_maps = []
    for c in range(NCORES):
        p = c % NP
        in_maps.append({
            "xf": xf, "cgi": chi_gidx_cores[c], "wgi": wil_gidx_p[p],
            "ogi": omg_gidx_p[p], "wchi": wchi, "womg": womg,
        })
    r = run_bass_kernel_spmd(nc, in_maps, core_ids=list(range(NCORES)),
                             trace=_want_trace)
    out = np.zeros(B, np.complex64)
    for c in range(NCORES):
        g, p = c // NP, c % NP
        o = r.results[c]["out"]
        v = (o[:, 0:4] + 1j * o[:, 4:8]).reshape(16, 4, 4)  # [b_lo, o, bh]
        vv = v.sum(axis=1)                                  # [b_lo, bh]
        for bh in range(4):
            out[g * 64 + bh * 16:g * 64 + bh * 16 + 16] += vv[:, bh]
    kernel._LAST_R = r
    if _want_trace:
        kernel._last_result = r
    return out


# revision 35
# speedup vs baseline: 1.0891x; 1.0891x over previous
"""Trainium2 Bass kernel for nn_ApproxSymmetricNet (gnn_message_passing).

8 NeuronCores, hybrid sharding: core c = (g=c//4 batch-half, p=c%4
plaquette-quarter).

chi is SITE-sharded by 8 (2048 sites x all 128 batches per core; 9 gather
descriptors per site). wilson/omega are (batch-half x plaquette-quarter)
sharded: rows hold 64 batches -> 512 fp16 = 1KB elements, so gather
descriptor counts drop to 4x4096 and 5x4096 per core (the SWDGE gather
ucode costs ~571ns + 7.86ns/descriptor of serial Pool time, independent
of element size -- the kernel's governing cost).

Dataflow: chi (gather x rows -> PE conv -> complex tanh -> PE transpose to
site-major fp16 rows [b64, ri, i]) -> dup-staged AllToAll (each dest quad
gets its batch-half) -> wilson (gather h1 rows, DVE products) -> quad
AllGather -> omega (gather h2 rows -> DRAM scratch -> HWDGE DMA-transpose
puts comps on partitions (PE transpose of gathered volume would be
PE-bound; transpose-mode dma_gather crashes this runtime) -> 5 f16
block-diag matmuls (K=128=(b_lo,ri,i) -> M=(ri_o,b_lo,o)) -> complex tanh
(reciprocal on ACT) -> free-dim reduce). Host sums channels + quarters.

tanh(x+iy) = (2T + i*(1-T^2)*sin2y) / D, D = 2*(1 - (1-T^2)*sin^2(y)).
"""
import numpy as np

import concourse.bacc as bacc
import concourse.mybir as mybir
import concourse.tile as tile
from concourse.bass_utils import run_bass_kernel_spmd

AFT = mybir.ActivationFunctionType
ALU = mybir.AluOpType
F32 = mybir.dt.float32
F16 = mybir.dt.float16
I16 = mybir.dt.int16

B, N_SITES, N_PLAQ = 128, 16384, 16384
K_CHI, P_SZ, K_OMG = 9, 4, 5
C_CHI, C_OMG = 4, 4
WILSON_RESCALE = 10 ** 1.5
NCORES = 8
NG, NP = 2, 4               # batch groups x plaquette quarters
BG = B // NG                # 64 batches per group
S_LOC = N_SITES // NCORES   # 2048 sites per core (chi)
P_LOC = N_PLAQ // NP        # 4096 plaquettes per core (wilson/omega)

# chi chunking (site-sharded)
DN = 14                     # sites per partition-group (14*9=126 partitions)
CHI_COLS = 4
CHI_SITES = DN * CHI_COLS   # 56 sites per chunk
CHI_NCH = (S_LOC + CHI_SITES - 1) // CHI_SITES      # 37

NQ = 2                      # a2a site-halves
SQ = S_LOC // NQ
WIL_PC = 256                # wilson plaquettes per chunk (4*256=1024 idx)
WIL_NCH = P_LOC // WIL_PC   # 16
OMG_PC = 1024               # omega plaquettes per chunk (5 calls of 1024)
OMG_NCH = P_LOC // OMG_PC   # 4
AG_PC = 1024                # h2 AllGather granularity (4 chunks)
DEBUG_DUMP = False


def _wrap_idx16(flat):
    n = len(flat)
    a = flat.reshape(n // 16, 16).T
    return np.tile(a, (8, 1)).astype(np.int16)


def _h1row(n):
    # site n -> row in d_recv [2 q][8 src][1024]
    return (n % S_LOC) // SQ * (NCORES * SQ) + (n // S_LOC) * SQ + n % SQ


def _h2row(m):
    # plaquette m -> row in d_h2g [4j][4rank][1024]
    return (m % P_LOC) // AG_PC * (NP * AG_PC) + (m // P_LOC) * AG_PC \
        + m % AG_PC


def build_host_tables(chi_kernel_idx, plaquette_idx, omega_kernel_idx,
                      chi_w, omega_w):
    # ---- per-core chi gather tables (site shard c) ----
    ci = np.concatenate(
        [chi_kernel_idx, np.full((CHI_SITES, K_CHI), N_SITES, np.int64)])
    chi_gidx_cores = []
    j = np.arange(CHI_NCH * CHI_COLS * 128)
    col = j // 128
    pp = j % 128
    dn = pp // K_CHI
    k = pp % K_CHI
    nl = (j // (CHI_COLS * 128)) * CHI_SITES + col % CHI_COLS * DN \
        + np.minimum(dn, DN - 1)
    for cc in range(NCORES):
        n = np.where(nl < S_LOC, cc * S_LOC + nl, N_SITES)
        flat = ci[np.minimum(n, N_SITES), k]
        flat[pp >= DN * K_CHI] = 0
        chi_gidx_cores.append(_wrap_idx16(flat))

    # ---- per-quarter wilson gather tables ----
    wil_gidx_p = []
    for p in range(NP):
        flat = np.zeros(WIL_NCH * 4 * WIL_PC, np.int64)
        for ch in range(WIL_NCH):
            jj = np.arange(4 * WIL_PC)
            kk = jj // WIL_PC
            cc_ = (jj % WIL_PC) // 128
            qq = jj % 128
            m = p * P_LOC + ch * WIL_PC + cc_ * 128 + qq
            flat[ch * 4 * WIL_PC + jj] = _h1row(plaquette_idx[m, kk])
        wil_gidx_p.append(_wrap_idx16(flat))

    # ---- per-quarter omega gather tables ----
    omg_gidx_p = []
    for p in range(NP):
        flat = np.zeros(OMG_NCH * K_OMG * OMG_PC, np.int64)
        pos = 0
        for ch in range(OMG_NCH):
            for k in range(K_OMG):
                m = p * P_LOC + ch * OMG_PC + np.arange(OMG_PC)
                flat[pos:pos + OMG_PC] = _h2row(omega_kernel_idx[m, k])
                pos += OMG_PC
        omg_gidx_p.append(_wrap_idx16(flat))

    # ---- chi weight lhsT [128,128]: row (dn*9+k) -> col (ri*64+dn*4+i) ----
    wchi = np.zeros((128, 128), np.float32)
    for dn_ in range(DN):
        for k in range(K_CHI):
            for i in range(C_CHI):
                wchi[dn_ * K_CHI + k, 0 * 64 + dn_ * 4 + i] = \
                    chi_w[i, 0, k].real
                wchi[dn_ * K_CHI + k, 1 * 64 + dn_ * 4 + i] = \
                    chi_w[i, 0, k].imag

    # ---- omega weights [128, 5*128] f16:
    # row (b_lo*8 + ri*4 + i) -> col (ri_o*64 + b_lo*4 + o) ----
    woms = []
    for k in range(K_OMG):
        w = np.zeros((128, 128), np.float32)
        for bl in range(16):
            for i in range(C_CHI):
                for o in range(C_OMG):
                    wr = omega_w[o, i, k].real
                    wi = omega_w[o, i, k].imag
                    w[bl * 8 + 0 * 4 + i, 0 * 64 + bl * 4 + o] = wr
                    w[bl * 8 + 1 * 4 + i, 0 * 64 + bl * 4 + o] = -wi
                    w[bl * 8 + 0 * 4 + i, 1 * 64 + bl * 4 + o] = wi
                    w[bl * 8 + 1 * 4 + i, 1 * 64 + bl * 4 + o] = wr
        woms.append(w)
    womg = np.concatenate(woms, axis=1).astype(np.float16)
    return chi_gidx_cores, wil_gidx_p, omg_gidx_p, wchi, womg


def emit_ctanh(nc, pool, pslist, out_re, out_im, tagpfx, FG=1):
    """Complex tanh of psum (re, im) pairs stacked in partitions AND free.

    pslist entries: (sx, sy, u, fg): psum [H, F0] placed at partitions
    [u*64, u*64+H), free [fg*F0, (fg+1)*F0). out_re/out_im [P, FG*F0].
    ACT ops grouped by function to limit table reloads; DVE f16 +
    reciprocal.
    """
    H = pslist[0][0].shape[0]
    P = out_re.shape[0]
    F = out_re.free_size()
    F0 = F // FG

    def t(nm):
        return pool.tile([P, F], F16, name=f"{tagpfx}_{nm}", tag=f"ct_{nm}",
                         bufs=2)
    T_, s_, c_, t2, q_, u_, d_, r_ = (t(x) for x in
                                      ("T", "s", "c", "t2", "q", "u", "d",
                                       "r"))

    def sl2(u, fg):
        return (slice(u * 64, u * 64 + H),
                slice(fg * F0, (fg + 1) * F0))
    for (sx, sy, u, fg) in pslist:
        nc.scalar.activation(T_[sl2(u, fg)], sx, AFT.Tanh)
    for (sx, sy, u, fg) in pslist:
        nc.scalar.activation(s_[sl2(u, fg)], sy, AFT.Sin, scale=2.0)
    for (sx, sy, u, fg) in pslist:
        nc.scalar.activation(c_[sl2(u, fg)], sy, AFT.Sin)
    nc.vector.tensor_mul(t2[:], T_[:], T_[:])
    nc.vector.tensor_scalar(out=q_[:], in0=t2[:], scalar1=-1.0, scalar2=1.0,
                            op0=ALU.mult, op1=ALU.add)
    nc.vector.tensor_mul(u_[:], c_[:], c_[:])
    nc.vector.tensor_mul(u_[:], u_[:], q_[:])
    nc.vector.tensor_scalar(out=d_[:], in0=u_[:], scalar1=-2.0, scalar2=2.0,
                            op0=ALU.mult, op1=ALU.add)
    nc.vector.reciprocal(r_[:], d_[:])
    nc.vector.scalar_tensor_tensor(out=out_re, in0=T_[:], scalar=2.0,
                                   in1=r_[:], op0=ALU.mult, op1=ALU.mult)
    nc.vector.tensor_mul(s_[:], s_[:], q_[:])
    nc.vector.tensor_mul(out_im, s_[:], r_[:])


def emit_taylor_ctanh(nc, pool, za, zb, out_re, out_im):
    """tanh(w) ~= w - w^3/3 = w*(1 - w^2/3) for |w| <~ 0.2 (omega's z is
    <~ 0.05 for this model scale; validated host-side). DVE-only: frees the
    ACT engine and psum banks quickly. za/zb: psum [128,512] with re in
    partitions [0:64], im in [64:128]; stacked as two units."""
    def t(nm):
        return pool.tile([128, 512], F16, name=f"tl_{nm}", tag=f"tl_{nm}",
                         bufs=2)
    zx, zy, x2, y2, q2, p_, pr, t1, v1, v3, v4 = (
        t(x) for x in ("zx", "zy", "x2", "y2", "q2", "p", "pr", "t1", "v1",
                       "v3", "v4"))
    nc.vector.tensor_copy(out=zx[0:64, :], in_=za[0:64, :])
    nc.vector.tensor_copy(out=zx[64:128, :], in_=zb[0:64, :])
    nc.vector.tensor_copy(out=zy[0:64, :], in_=za[64:128, :])
    nc.vector.tensor_copy(out=zy[64:128, :], in_=zb[64:128, :])
    nc.vector.tensor_mul(x2[:], zx[:], zx[:])
    nc.vector.tensor_mul(y2[:], zy[:], zy[:])
    nc.vector.tensor_mul(q2[:], zx[:], zy[:])
    nc.vector.scalar_tensor_tensor(out=p_[:], in0=y2[:], scalar=-1.0,
                                   in1=x2[:], op0=ALU.mult, op1=ALU.add)
    nc.vector.tensor_scalar(out=pr[:], in0=p_[:], scalar1=-1.0 / 3.0,
                            scalar2=1.0, op0=ALU.mult, op1=ALU.add)
    nc.vector.tensor_mul(t1[:], zx[:], pr[:])
    nc.vector.tensor_mul(v1[:], zy[:], q2[:])
    nc.vector.scalar_tensor_tensor(out=out_re, in0=v1[:], scalar=2.0 / 3.0,
                                   in1=t1[:], op0=ALU.mult, op1=ALU.add)
    nc.vector.tensor_mul(v3[:], zx[:], q2[:])
    nc.vector.tensor_mul(v4[:], zy[:], pr[:])
    nc.vector.scalar_tensor_tensor(out=out_im, in0=v3[:], scalar=-2.0 / 3.0,
                                   in1=v4[:], op0=ALU.mult, op1=ALU.add)


def build_kernel():
    nc = bacc.Bacc("TRN2", target_bir_lowering=False, debug=True)

    d_xf = nc.dram_tensor("xf", [N_SITES + 1, 128], F32, kind="ExternalInput")
    d_cgi = nc.dram_tensor("cgi", [128, CHI_NCH * CHI_COLS * 8], I16,
                           kind="ExternalInput")
    d_wgi = nc.dram_tensor("wgi", [128, WIL_NCH * 64], I16,
                           kind="ExternalInput")
    d_ogi = nc.dram_tensor("ogi", [128, OMG_NCH * K_OMG * (OMG_PC // 16)], I16,
                           kind="ExternalInput")
    d_wchi = nc.dram_tensor("wchi", [128, 128], F32, kind="ExternalInput")
    d_womg = nc.dram_tensor("womg", [128, K_OMG * 128], F16,
                            kind="ExternalInput")
    # a2a staging: per site-half, 8 dest blocks of [1024 sites, 512 f16]
    d_stq = [nc.dram_tensor(f"stq{q}", [NCORES, SQ, 512], F16)
             for q in range(NQ)]
    # h1 for my batch group: rows q*8192 + src*1024 + loc
    d_recv = nc.dram_tensor("recv", [NQ, NCORES, SQ, 512], F16)
    # wilson output shard, j-major quarters of my 4096 plaquettes
    d_h2q = [nc.dram_tensor(f"h2q{j}", [AG_PC, 512], F16)
             for j in range(4)]
    # h2 full for my batch group: rows j*4096 + rank*1024 + loc
    d_h2g = nc.dram_tensor("h2g", [4, NP, AG_PC, 512], F16)
    if DEBUG_DUMP:
        d_recv_d = nc.dram_tensor("recv_d", [NQ * NCORES * SQ, 512], F16,
                                  kind="ExternalOutput")
        d_h2g_d = nc.dram_tensor("h2g_d", [4 * NP * AG_PC, 512], F16,
                                 kind="ExternalOutput")
        d_gt_d = nc.dram_tensor("gt_d", [128, OMG_PC], F16,
                                kind="ExternalOutput")
        d_zc_d = nc.dram_tensor("zc_d", [128, 512], F32,
                                kind="ExternalOutput")
        d_tr_d = nc.dram_tensor("tr_d", [128, 1024], F32,
                                kind="ExternalOutput")
    # omega gather scratch (double-buffered over chunks)
    d_gsc = nc.dram_tensor("gsc", [OMG_NCH, K_OMG, OMG_PC, 512], F16)
    d_out = nc.dram_tensor("out", [64, 8], F32, kind="ExternalOutput")

    with tile.TileContext(nc) as tc, \
         nc.allow_low_precision(reason="fp16 rows/tanh; tol 2e-2"):
        with tc.tile_pool(name="pidx", bufs=1) as pidx, \
             tc.tile_pool(name="pwork", bufs=1) as pool, \
             tc.tile_pool(name="ppsum", bufs=1, space="PSUM") as ppsum:
            t_cgi = pidx.tile([128, CHI_NCH * CHI_COLS * 8], I16,
                              name="t_cgi")
            t_wgi = pidx.tile([128, WIL_NCH * 64], I16, name="t_wgi")
            t_ogi = pidx.tile([128, OMG_NCH * K_OMG * (OMG_PC // 16)], I16,
                              name="t_ogi")
            t_wchi = pidx.tile([128, 128], F32, name="t_wchi")
            t_womg = pidx.tile([128, K_OMG * 128], F16, name="t_womg")
            nc.sync.dma_start(t_cgi[:], d_cgi[:])
            nc.sync.dma_start(t_wgi[:], d_wgi[:])
            nc.sync.dma_start(t_ogi[:], d_ogi[:])
            nc.sync.dma_start(t_wchi[:], d_wchi[:])
            nc.sync.dma_start(t_womg[:], d_womg[:])

            # =========== chi (site-sharded, all 128 batches) ===========
            # h1 row content is [i(4), ri(2), b(64)]: the stage write from
            # ctanh's [(dn,i) parts, (col,b)] layout is then a 3-dim AP with
            # 128B runs: merged (dn,i) stride 128-els, col stride 7168, b 1.
            q_written = [0] * NQ

            def stage_write(h1re, h1im, npair, pr):
                nonlocal dma_flip
                for u in range(npair):
                    s0 = (pr + u) * CHI_SITES
                    # rectangles (quarter, col0, ncol, dn0, ndn) not crossing
                    # a quarter boundary nor S_LOC
                    pieces = []
                    for colq in range(CHI_COLS):
                        base = s0 + colq * DN
                        ns = min(DN, max(0, S_LOC - base))
                        st = 0
                        while st < ns:
                            s = base + st
                            qh = s // SQ
                            n_ = min(ns - st, (qh + 1) * SQ - s)
                            pieces.append((qh, colq, st, n_))
                            st += n_
                    rects = []
                    ii = 0
                    while ii < len(pieces):
                        qh, colq, st, n_ = pieces[ii]
                        if st == 0 and n_ == DN:
                            jj = ii
                            while (jj + 1 < len(pieces)
                                   and pieces[jj + 1][:1] == (qh,)
                                   and pieces[jj + 1][2] == 0
                                   and pieces[jj + 1][3] == DN
                                   and pieces[jj + 1][1] == pieces[jj][1] + 1):
                                jj += 1
                            rects.append((qh, colq, pieces[jj][1] - colq + 1,
                                          0, DN))
                            ii = jj + 1
                        else:
                            rects.append((qh, colq, 1, st, n_))
                            ii += 1
                    for (qh, c0, ncol, dn0, ndn) in rects:
                        # group index m = local_site*4 + i; multi-col rects
                        # are always full cols (dn0=0, ndn=DN)
                        m0 = (s0 - qh * SQ + c0 * DN + dn0) * 4
                        for ri, tl in ((0, h1re), (1, h1im)):
                            for gg in range(NG):
                                M = d_stq[qh][gg * NP].rearrange(
                                    "s (m x) -> (s m) x", m=4)
                                if ncol == 1:
                                    dst = M[m0:m0 + 4 * ndn,
                                            ri * 64:(ri + 1) * 64]
                                else:
                                    dst = M[m0:m0 + ncol * 4 * DN, :] \
                                        .rearrange("(c m) x -> m c x",
                                                   c=ncol)[
                                        :, :, ri * 64:(ri + 1) * 64]
                                src = tl[u * 64 + dn0 * 4:
                                         u * 64 + (dn0 + ndn) * 4,
                                         c0:c0 + ncol, gg * 64:(gg + 1) * 64]
                                if ncol == 1:
                                    src = src.rearrange("p c b -> p (c b)")
                                eng = nc.sync if dma_flip else nc.scalar
                                dma_flip = not dma_flip
                                eng.dma_start(dst, src)
                        q_written[qh] += ncol * ndn

            dups_done = 0
            dma_flip = True
            for base in range(0, CHI_NCH, 4):
                nch = min(4, CHI_NCH - base)
                pss = []
                for u in range(nch):
                    ch = base + u
                    gch = pool.tile([128, CHI_COLS, 128], F32, name="gchi",
                                    tag="gchi", bufs=5)
                    nc.gpsimd.dma_gather(
                        gch[:], d_xf[:],
                        t_cgi[:, ch * CHI_COLS * 8:(ch + 1) * CHI_COLS * 8],
                        CHI_COLS * 128, CHI_COLS * 128, 128, elem_step=128)
                    pch = ppsum.tile([128, 512], F32, name="pchi", tag="pchi",
                                     bufs=4)
                    nc.tensor.matmul(
                        pch[:], lhsT=t_wchi[:],
                        rhs=gch[:].rearrange("p a b -> p (a b)"),
                        start=True, stop=True)
                    pss.append(pch)
                h1re = pool.tile([120, 2, CHI_COLS, 128], F16, name="h1re",
                                 tag="h1re", bufs=3)
                h1im = pool.tile([120, 2, CHI_COLS, 128], F16, name="h1im",
                                 tag="h1im", bufs=3)
                emit_ctanh(
                    nc, pool,
                    [(p_[0:56, :], p_[64:120, :], u % 2, u // 2)
                     for u, p_ in enumerate(pss)],
                    h1re[:].rearrange("p g a b -> p (g a b)"),
                    h1im[:].rearrange("p g a b -> p (g a b)"), "ctchi",
                    FG=2)
                for pg in range(2):
                    npair = min(2, nch - 2 * pg)
                    if npair <= 0:
                        break
                    stage_write(h1re[:, pg], h1im[:, pg], npair,
                                base + 2 * pg)
                # fire dup copies + a2a per completed site-half
                while dups_done < NQ and q_written[dups_done] >= SQ:
                    qd = dups_done
                    for gg in range(NG):
                        for pp_ in range(1, NP):
                            eng = nc.sync if dma_flip else nc.scalar
                            dma_flip = not dma_flip
                            eng.dma_start(
                                d_stq[qd][gg * NP + pp_], d_stq[qd][gg * NP])
                    nc.gpsimd.collective_compute(
                        "AllToAll", ALU.bypass,
                        replica_groups=[list(range(NCORES))],
                        ins=[d_stq[qd][:]], outs=[d_recv[qd]])
                    dups_done += 1

            # =========== wilson (quarter p, my 64 batches) ===========
            recv_flat = d_recv[:].rearrange("a b c f -> (a b c) f")
            for ch in range(WIL_NCH):
                gw = pool.tile([128, 4, 2, 512], F16, name="gwil", tag="gwil",
                               bufs=3)
                nc.gpsimd.dma_gather(
                    gw[:].rearrange("p a b f -> p (a b) f"), recv_flat,
                    t_wgi[:, ch * 64:(ch + 1) * 64], 4 * WIL_PC, 4 * WIL_PC,
                    512, elem_step=512)
                m1 = pool.tile([128, 2, 512], F16, name="wm1", tag="wm1",
                               bufs=2)
                m2 = pool.tile([128, 2, 512], F16, name="wm2", tag="wm2",
                               bufs=2)
                h2t = pool.tile([128, 2, 512], F16, name="h2t", tag="h2t",
                                bufs=2)
                nc.vector.tensor_mul(m1[:], gw[:, 0, :, :], gw[:, 1, :, :])
                nc.vector.tensor_mul(m2[:], gw[:, 2, :, :], gw[:, 3, :, :])
                # content reorder [i,ri,b] -> [b,ri,i] for omega's
                # K-slices; DVE APs are limited to 3D, so split by (cc, ri)
                for cc in range(2):
                    for ri in range(2):
                        dstv = h2t[:, cc, :].rearrange(
                            "p (b r i) -> r p b i", b=64, r=2, i=4)[ri]
                        nc.vector.scalar_tensor_tensor(
                            out=dstv,
                            in0=m1[:, cc, :].rearrange(
                                "p (i r b) -> r p b i", i=4, r=2, b=64)[ri],
                            scalar=float(WILSON_RESCALE),
                            in1=m2[:, cc, :].rearrange(
                                "p (i r b) -> r p b i", i=4, r=2, b=64)[ri],
                            op0=ALU.mult, op1=ALU.mult)
                jq, r0 = ch // 4, (ch % 4) * WIL_PC
                nc.sync.dma_start(
                    d_h2q[jq][r0:r0 + WIL_PC, :]
                    .rearrange("(c q) f -> q c f", c=2), h2t[:])
                if ch % 4 == 3:
                    nc.gpsimd.collective_compute(
                        "AllGather", ALU.bypass,
                        replica_groups=[[0, 1, 2, 3], [4, 5, 6, 7]],
                        ins=[d_h2q[jq][:]], outs=[d_h2g[jq]])

            # =========== omega (quarter p, my 64 batches) ===========
            h2g_flat = d_h2g[:].rearrange("a b c f -> (a b c) f")
            acc_re = pool.tile([64, 4], F32, name="acc_re")
            acc_im = pool.tile([64, 4], F32, name="acc_im")
            nc.vector.memset(acc_re[:], 0.0)
            nc.vector.memset(acc_im[:], 0.0)

            def omega_tail(zcs, grp):
                # zcs[j]: bh = 2*grp + j//2, half = j%2; pair (pr_, pr_+2)
                # stacks bh 2*grp (rows 0:64) with bh 2*grp+1 (rows 64:128)
                for pr_ in range(2):
                    za, zb = zcs[pr_], zcs[pr_ + 2]
                    tr = pool.tile([128, 512], F32, name="otr", tag="otr",
                                   bufs=2)
                    ti = pool.tile([128, 512], F32, name="oti", tag="oti",
                                   bufs=2)
                    emit_taylor_ctanh(nc, pool, za, zb, tr[:], ti[:])
                    rr = pool.tile([128, 1], F32, name="orr", tag="orr",
                                   bufs=2)
                    rim = pool.tile([128, 1], F32, name="ori", tag="ori",
                                    bufs=2)
                    nc.vector.tensor_reduce(
                        out=rr[:], in_=tr[:], axis=mybir.AxisListType.X,
                        op=ALU.add)
                    nc.vector.tensor_reduce(
                        out=rim[:], in_=ti[:], axis=mybir.AxisListType.X,
                        op=ALU.add)
                    bha, bhb = 2 * grp, 2 * grp + 1
                    nc.vector.tensor_add(
                        acc_re[:, bha:bha + 1], acc_re[:, bha:bha + 1],
                        rr[0:64, :])
                    nc.vector.tensor_add(
                        acc_im[:, bha:bha + 1], acc_im[:, bha:bha + 1],
                        rim[0:64, :])
                    r2 = pool.tile([64, 1], F32, name="or2", tag="or2",
                                   bufs=2)
                    i2 = pool.tile([64, 1], F32, name="oi2", tag="oi2",
                                   bufs=2)
                    nc.vector.tensor_copy(out=r2[:], in_=rr[64:128, :])
                    nc.vector.tensor_copy(out=i2[:], in_=rim[64:128, :])
                    nc.vector.tensor_add(
                        acc_re[:, bhb:bhb + 1], acc_re[:, bhb:bhb + 1],
                        r2[:])
                    nc.vector.tensor_add(
                        acc_im[:, bhb:bhb + 1], acc_im[:, bhb:bhb + 1],
                        i2[:])

            for ch in range(OMG_NCH):
                buf = ch
                for k in range(K_OMG):
                    gk = pool.tile([128, OMG_PC // 128, 512], F16,
                                   name="gomg", tag="gomg", bufs=5)
                    off = (ch * K_OMG + k) * (OMG_PC // 16)
                    nc.gpsimd.dma_gather(
                        gk[:], h2g_flat, t_ogi[:, off:off + OMG_PC // 16],
                        OMG_PC, OMG_PC, 512, elem_step=512)
                    nc.scalar.dma_start(
                        d_gsc[buf, k].rearrange("(c q) f -> q c f",
                                                c=OMG_PC // 128),
                        gk[:])
                for grp in range(2):
                    zcs = [ppsum.tile([128, 512], F32, name=f"zc{jj}",
                                      tag=f"zc{jj}", bufs=1)
                           for jj in range(4)]
                    for k in range(K_OMG):
                        gT = pool.tile([128, 2, OMG_PC], F16, name="gT",
                                       tag="gT", bufs=5)
                        for bi in range(2):
                            bh = 2 * grp + bi
                            nc.sync.dma_start_transpose(
                                gT[:, bi, :],
                                d_gsc[buf, k][:, 128 * bh:128 * (bh + 1)])
                        for bi in range(2):
                            for hh in range(2):
                                nc.tensor.matmul(
                                    zcs[bi * 2 + hh][:],
                                    lhsT=t_womg[:, k * 128:(k + 1) * 128],
                                    rhs=gT[:, bi, 512 * hh:512 * (hh + 1)],
                                    start=(k == 0), stop=(k == K_OMG - 1))
                    omega_tail(zcs, grp)

            if DEBUG_DUMP:
                nc.sync.dma_start(
                    d_recv_d[:],
                    d_recv[:].rearrange("a b c f -> (a b c) f"))
                nc.sync.dma_start(
                    d_h2g_d[:],
                    d_h2g[:].rearrange("a b c f -> (a b c) f"))
            out_t = pool.tile([64, 8], F32, name="out_t")
            nc.vector.tensor_copy(out=out_t[:, 0:4], in_=acc_re[:])
            nc.vector.tensor_copy(out=out_t[:, 4:8], in_=acc_im[:])
            nc.sync.dma_start(d_out[:], out_t[:])
    nc.compile()
    return nc


_NC_CACHE = None


def kernel(x, chi_kernel_idx, chi_kernel_mask, plaquette_idx, plaquette_mask,
           omega_kernel_idx, omega_kernel_mask, chi_w, chi_b, omega_w,
           omega_b, _want_trace=False):
    global _NC_CACHE
    x = np.asarray(x, np.float32)
    chi_kernel_idx = np.asarray(chi_kernel_idx).astype(np.int64)
    plaquette_idx = np.asarray(plaquette_idx).astype(np.int64)
    omega_kernel_idx = np.asarray(omega_kernel_idx).astype(np.int64)
    chi_w = np.asarray(chi_w)
    omega_w = np.asarray(omega_w)

    chi_gidx_cores, wil_gidx_p, omg_gidx_p, wchi, womg = build_host_tables(
        chi_kernel_idx, plaquette_idx, omega_kernel_idx, chi_w, omega_w)

    # omega uses a cubic Taylor tanh; verify |z_omega| is in-range
    _h1 = np.tanh(np.einsum(
        "bnk,ik->bni", x.astype(np.complex64)[:, chi_kernel_idx],
        chi_w[:, 0, :]))
    _gp = _h1[:, plaquette_idx, :]
    _h2 = (WILSON_RESCALE * _gp.real.prod(axis=2)
           + 1j * WILSON_RESCALE * _gp.imag.prod(axis=2))
    _z = np.einsum("bnki,oik->bno", _h2[:, omega_kernel_idx, :], omega_w)
    _zmax = max(np.abs(_z.real).max(), np.abs(_z.imag).max())
    assert _zmax < 0.25, f"omega pre-tanh out of Taylor range: {_zmax}"

    if _NC_CACHE is None:
        _NC_CACHE = build_kernel()
    nc = _NC_CACHE

    xf = np.zeros((N_SITES + 1, 128), np.float32)
    xf[:N_SITES] = x.T
    in_maps = []
    for c in range(NCORES):
        p = c % NP
        in_maps.append({
            "xf": xf, "cgi": chi_gidx_cores[c], "wgi": wil_gidx_p[p],
            "ogi": omg_gidx_p[p], "wchi": wchi, "womg": womg,
        })
    r = run_bass_kernel_spmd(nc, in_maps, core_ids=list(range(NCORES)),
                             trace=_want_trace)
    out = np.zeros(B, np.complex64)
    for c in range(NCORES):
        g, p = c // NP, c % NP
        o = r.results[c]["out"]
        v = (o[:, 0:4] + 1j * o[:, 4:8]).reshape(16, 4, 4)  # [b_lo, o, bh]
        vv = v.sum(axis=1)                                  # [b_lo, bh]
        for bh in range(4):
            out[g * 64 + bh * 16:g * 64 + bh * 16 + 16] += vv[:, bh]
    kernel._LAST_R = r
    if _want_trace:
        kernel._last_result = r
    return out


# revision 36
# speedup vs baseline: 1.1314x; 1.0388x over previous
"""Trainium2 Bass kernel for nn_ApproxSymmetricNet (gnn_message_passing).

8 NeuronCores, hybrid sharding: core c = (g=c//4 batch-half, p=c%4
plaquette-quarter).

chi is SITE-sharded by 8 (2048 sites x all 128 batches per core; 9 gather
descriptors per site). wilson/omega are (batch-half x plaquette-quarter)
sharded: rows hold 64 batches -> 512 fp16 = 1KB elements, so gather
descriptor counts drop to 4x4096 and 5x4096 per core (the SWDGE gather
ucode costs ~571ns + 7.86ns/descriptor of serial Pool time, independent
of element size -- the kernel's governing cost).

Dataflow: chi (gather x rows -> PE conv -> complex tanh -> PE transpose to
site-major fp16 rows [b64, ri, i]) -> dup-staged AllToAll (each dest quad
gets its batch-half) -> wilson (gather h1 rows, DVE products) -> quad
AllGather -> omega (gather h2 rows -> DRAM scratch -> HWDGE DMA-transpose
puts comps on partitions (PE transpose of gathered volume would be
PE-bound; transpose-mode dma_gather crashes this runtime) -> 5 f16
block-diag matmuls (K=128=(b_lo,ri,i) -> M=(ri_o,b_lo,o)) -> complex tanh
(reciprocal on ACT) -> free-dim reduce). Host sums channels + quarters.

tanh(x+iy) = (2T + i*(1-T^2)*sin2y) / D, D = 2*(1 - (1-T^2)*sin^2(y)).
"""
import numpy as np

import concourse.bacc as bacc
import concourse.mybir as mybir
import concourse.tile as tile
from concourse.bass_utils import run_bass_kernel_spmd

AFT = mybir.ActivationFunctionType
ALU = mybir.AluOpType
F32 = mybir.dt.float32
F16 = mybir.dt.float16
I16 = mybir.dt.int16

B, N_SITES, N_PLAQ = 128, 16384, 16384
K_CHI, P_SZ, K_OMG = 9, 4, 5
C_CHI, C_OMG = 4, 4
WILSON_RESCALE = 10 ** 1.5
NCORES = 8
NG, NP = 2, 4               # batch groups x plaquette quarters
BG = B // NG                # 64 batches per group
S_LOC = N_SITES // NCORES   # 2048 sites per core (chi)
P_LOC = N_PLAQ // NP        # 4096 plaquettes per core (wilson/omega)

# chi chunking (site-sharded)
DN = 14                     # sites per partition-group (14*9=126 partitions)
CHI_COLS = 4
CHI_SITES = DN * CHI_COLS   # 56 sites per chunk
CHI_NCH = (S_LOC + CHI_SITES - 1) // CHI_SITES      # 37

NQ = 2                      # a2a site-halves
SQ = S_LOC // NQ
WIL_PC = 256                # wilson plaquettes per chunk (4*256=1024 idx)
WIL_NCH = P_LOC // WIL_PC   # 16
OMG_PC = 1024               # omega plaquettes per chunk (5 calls of 1024)
OMG_NCH = P_LOC // OMG_PC   # 4
AG_PC = 1024                # h2 AllGather granularity (4 chunks)
DEBUG_DUMP = False


def _wrap_idx16(flat):
    n = len(flat)
    a = flat.reshape(n // 16, 16).T
    return np.tile(a, (8, 1)).astype(np.int16)


def _h1row(n):
    # site n -> row in d_recv [2 q][8 src][1024]
    return (n % S_LOC) // SQ * (NCORES * SQ) + (n // S_LOC) * SQ + n % SQ


def _h2row(m):
    # plaquette m -> row in d_h2g [4j][4rank][1024]
    return (m % P_LOC) // AG_PC * (NP * AG_PC) + (m // P_LOC) * AG_PC \
        + m % AG_PC


def build_host_tables(chi_kernel_idx, plaquette_idx, omega_kernel_idx,
                      chi_w, omega_w):
    # ---- per-core chi gather tables (site shard c) ----
    ci = np.concatenate(
        [chi_kernel_idx, np.full((CHI_SITES, K_CHI), N_SITES, np.int64)])
    chi_gidx_cores = []
    j = np.arange(CHI_NCH * CHI_COLS * 128)
    col = j // 128
    pp = j % 128
    dn = pp // K_CHI
    k = pp % K_CHI
    nl = (j // (CHI_COLS * 128)) * CHI_SITES + col % CHI_COLS * DN \
        + np.minimum(dn, DN - 1)
    for cc in range(NCORES):
        n = np.where(nl < S_LOC, cc * S_LOC + nl, N_SITES)
        flat = ci[np.minimum(n, N_SITES), k]
        flat[pp >= DN * K_CHI] = 0
        chi_gidx_cores.append(_wrap_idx16(flat))

    # ---- per-quarter wilson gather tables ----
    wil_gidx_p = []
    for p in range(NP):
        flat = np.zeros(WIL_NCH * 4 * WIL_PC, np.int64)
        for ch in range(WIL_NCH):
            jj = np.arange(4 * WIL_PC)
            kk = jj // WIL_PC
            cc_ = (jj % WIL_PC) // 128
            qq = jj % 128
            m = p * P_LOC + ch * WIL_PC + cc_ * 128 + qq
            flat[ch * 4 * WIL_PC + jj] = _h1row(plaquette_idx[m, kk])
        wil_gidx_p.append(_wrap_idx16(flat))

    # ---- per-quarter omega gather tables ----
    omg_gidx_p = []
    for p in range(NP):
        flat = np.zeros(OMG_NCH * K_OMG * OMG_PC, np.int64)
        pos = 0
        for ch in range(OMG_NCH):
            for k in range(K_OMG):
                m = p * P_LOC + ch * OMG_PC + np.arange(OMG_PC)
                flat[pos:pos + OMG_PC] = _h2row(omega_kernel_idx[m, k])
                pos += OMG_PC
        omg_gidx_p.append(_wrap_idx16(flat))

    # ---- chi weight lhsT [128,128]: row (dn*9+k) -> col (ri*64+dn*4+i) ----
    wchi = np.zeros((128, 128), np.float32)
    for dn_ in range(DN):
        for k in range(K_CHI):
            for i in range(C_CHI):
                wchi[dn_ * K_CHI + k, 0 * 64 + dn_ * 4 + i] = \
                    chi_w[i, 0, k].real
                wchi[dn_ * K_CHI + k, 1 * 64 + dn_ * 4 + i] = \
                    chi_w[i, 0, k].imag

    # ---- omega weights [128, 5*128] f16:
    # row (b_lo*8 + ri*4 + i) -> col (ri_o*64 + b_lo*4 + o) ----
    woms = []
    for k in range(K_OMG):
        w = np.zeros((128, 128), np.float32)
        for bl in range(16):
            for i in range(C_CHI):
                for o in range(C_OMG):
                    wr = omega_w[o, i, k].real
                    wi = omega_w[o, i, k].imag
                    w[bl * 8 + 0 * 4 + i, 0 * 64 + bl * 4 + o] = wr
                    w[bl * 8 + 1 * 4 + i, 0 * 64 + bl * 4 + o] = -wi
                    w[bl * 8 + 0 * 4 + i, 1 * 64 + bl * 4 + o] = wi
                    w[bl * 8 + 1 * 4 + i, 1 * 64 + bl * 4 + o] = wr
        woms.append(w)
    womg = np.concatenate(woms, axis=1).astype(np.float16)
    return chi_gidx_cores, wil_gidx_p, omg_gidx_p, wchi, womg


def emit_ctanh(nc, pool, pslist, out_re, out_im, tagpfx, FG=1):
    """Complex tanh of psum (re, im) pairs stacked in partitions AND free.

    pslist entries: (sx, sy, u, fg): psum [H, F0] placed at partitions
    [u*64, u*64+H), free [fg*F0, (fg+1)*F0). out_re/out_im [P, FG*F0].
    ACT ops grouped by function to limit table reloads; DVE f16 +
    reciprocal.
    """
    H = pslist[0][0].shape[0]
    P = out_re.shape[0]
    F = out_re.free_size()
    F0 = F // FG

    def t(nm):
        return pool.tile([P, F], F16, name=f"{tagpfx}_{nm}", tag=f"ct_{nm}",
                         bufs=2)
    T_, s_, c_, t2, q_, u_, d_, r_ = (t(x) for x in
                                      ("T", "s", "c", "t2", "q", "u", "d",
                                       "r"))

    def sl2(u, fg):
        return (slice(u * 64, u * 64 + H),
                slice(fg * F0, (fg + 1) * F0))
    for (sx, sy, u, fg) in pslist:
        nc.scalar.activation(T_[sl2(u, fg)], sx, AFT.Tanh)
    for (sx, sy, u, fg) in pslist:
        nc.scalar.activation(s_[sl2(u, fg)], sy, AFT.Sin, scale=2.0)
    for (sx, sy, u, fg) in pslist:
        nc.scalar.activation(c_[sl2(u, fg)], sy, AFT.Sin)
    nc.vector.tensor_mul(t2[:], T_[:], T_[:])
    nc.vector.tensor_scalar(out=q_[:], in0=t2[:], scalar1=-1.0, scalar2=1.0,
                            op0=ALU.mult, op1=ALU.add)
    nc.vector.tensor_mul(u_[:], c_[:], c_[:])
    nc.vector.tensor_mul(u_[:], u_[:], q_[:])
    nc.vector.tensor_scalar(out=d_[:], in0=u_[:], scalar1=-2.0, scalar2=2.0,
                            op0=ALU.mult, op1=ALU.add)
    nc.vector.reciprocal(r_[:], d_[:])
    nc.vector.scalar_tensor_tensor(out=out_re, in0=T_[:], scalar=2.0,
                                   in1=r_[:], op0=ALU.mult, op1=ALU.mult)
    nc.vector.tensor_mul(s_[:], s_[:], q_[:])
    nc.vector.tensor_mul(out_im, s_[:], r_[:])


def emit_taylor_ctanh(nc, pool, za, zb, out_re, out_im):
    """tanh(w) ~= w - w^3/3 = w*(1 - w^2/3) for |w| <~ 0.2 (omega's z is
    <~ 0.05 for this model scale; validated host-side). DVE-only: frees the
    ACT engine and psum banks quickly. za/zb: psum [128,512] with re in
    partitions [0:64], im in [64:128]; stacked as two units."""
    def t(nm):
        return pool.tile([128, 512], F16, name=f"tl_{nm}", tag=f"tl_{nm}",
                         bufs=2)
    zx, zy, x2, y2, q2, p_, pr, t1, v1, v3, v4 = (
        t(x) for x in ("zx", "zy", "x2", "y2", "q2", "p", "pr", "t1", "v1",
                       "v3", "v4"))
    nc.vector.tensor_copy(out=zx[0:64, :], in_=za[0:64, :])
    nc.vector.tensor_copy(out=zx[64:128, :], in_=zb[0:64, :])
    nc.vector.tensor_copy(out=zy[0:64, :], in_=za[64:128, :])
    nc.vector.tensor_copy(out=zy[64:128, :], in_=zb[64:128, :])
    nc.vector.tensor_mul(x2[:], zx[:], zx[:])
    nc.vector.tensor_mul(y2[:], zy[:], zy[:])
    nc.vector.tensor_mul(q2[:], zx[:], zy[:])
    nc.vector.scalar_tensor_tensor(out=p_[:], in0=y2[:], scalar=-1.0,
                                   in1=x2[:], op0=ALU.mult, op1=ALU.add)
    nc.vector.tensor_scalar(out=pr[:], in0=p_[:], scalar1=-1.0 / 3.0,
                            scalar2=1.0, op0=ALU.mult, op1=ALU.add)
    nc.vector.tensor_mul(t1[:], zx[:], pr[:])
    nc.vector.tensor_mul(v1[:], zy[:], q2[:])
    nc.vector.scalar_tensor_tensor(out=out_re, in0=v1[:], scalar=2.0 / 3.0,
                                   in1=t1[:], op0=ALU.mult, op1=ALU.add)
    nc.vector.tensor_mul(v3[:], zx[:], q2[:])
    nc.vector.tensor_mul(v4[:], zy[:], pr[:])
    nc.vector.scalar_tensor_tensor(out=out_im, in0=v3[:], scalar=-2.0 / 3.0,
                                   in1=v4[:], op0=ALU.mult, op1=ALU.add)


def build_kernel():
    nc = bacc.Bacc("TRN2", target_bir_lowering=False, debug=True)

    d_xf = nc.dram_tensor("xf", [N_SITES + 1, 128], F32, kind="ExternalInput")
    d_cgi = nc.dram_tensor("cgi", [128, CHI_NCH * CHI_COLS * 8], I16,
                           kind="ExternalInput")
    d_wgi = nc.dram_tensor("wgi", [128, WIL_NCH * 64], I16,
                           kind="ExternalInput")
    d_ogi = nc.dram_tensor("ogi", [128, OMG_NCH * K_OMG * (OMG_PC // 16)], I16,
                           kind="ExternalInput")
    d_wchi = nc.dram_tensor("wchi", [128, 128], F32, kind="ExternalInput")
    d_womg = nc.dram_tensor("womg", [128, K_OMG * 128], F16,
                            kind="ExternalInput")
    # a2a staging: per site-half, 8 dest blocks of [1024 sites, 512 f16]
    d_stq = [nc.dram_tensor(f"stq{q}", [NCORES, SQ, 512], F16)
             for q in range(NQ)]
    # h1 for my batch group: rows q*8192 + src*1024 + loc
    d_recv = nc.dram_tensor("recv", [NQ, NCORES, SQ, 512], F16)
    # wilson output shard, j-major quarters of my 4096 plaquettes
    d_h2q = [nc.dram_tensor(f"h2q{j}", [AG_PC, 512], F16)
             for j in range(4)]
    # h2 full for my batch group: rows j*4096 + rank*1024 + loc
    d_h2g = nc.dram_tensor("h2g", [4, NP, AG_PC, 512], F16)
    if DEBUG_DUMP:
        d_recv_d = nc.dram_tensor("recv_d", [NQ * NCORES * SQ, 512], F16,
                                  kind="ExternalOutput")
        d_h2g_d = nc.dram_tensor("h2g_d", [4 * NP * AG_PC, 512], F16,
                                 kind="ExternalOutput")
        d_gt_d = nc.dram_tensor("gt_d", [128, OMG_PC], F16,
                                kind="ExternalOutput")
        d_zc_d = nc.dram_tensor("zc_d", [128, 512], F32,
                                kind="ExternalOutput")
        d_tr_d = nc.dram_tensor("tr_d", [128, 1024], F32,
                                kind="ExternalOutput")
    # omega gather scratch (double-buffered over chunks)
    d_gsc = nc.dram_tensor("gsc", [OMG_NCH, K_OMG, OMG_PC, 512], F16)
    d_out = nc.dram_tensor("out", [64, 8], F32, kind="ExternalOutput")

    with tile.TileContext(nc) as tc, \
         nc.allow_low_precision(reason="fp16 rows/tanh; tol 2e-2"):
        with tc.tile_pool(name="pidx", bufs=1) as pidx, \
             tc.tile_pool(name="pwork", bufs=1) as pool, \
             tc.tile_pool(name="ppsum", bufs=1, space="PSUM") as ppsum:
            t_cgi = pidx.tile([128, CHI_NCH * CHI_COLS * 8], I16,
                              name="t_cgi")
            t_wgi = pidx.tile([128, WIL_NCH * 64], I16, name="t_wgi")
            t_ogi = pidx.tile([128, OMG_NCH * K_OMG * (OMG_PC // 16)], I16,
                              name="t_ogi")
            t_wchi = pidx.tile([128, 128], F32, name="t_wchi")
            t_womg = pidx.tile([128, K_OMG * 128], F16, name="t_womg")
            nc.sync.dma_start(t_cgi[:], d_cgi[:])
            nc.sync.dma_start(t_wgi[:], d_wgi[:])
            nc.sync.dma_start(t_ogi[:], d_ogi[:])
            nc.sync.dma_start(t_wchi[:], d_wchi[:])
            nc.sync.dma_start(t_womg[:], d_womg[:])

            # =========== chi (site-sharded, all 128 batches) ===========
            # h1 row content is [i(4), ri(2), b(64)]: the stage write from
            # ctanh's [(dn,i) parts, (col,b)] layout is then a 3-dim AP with
            # 128B runs: merged (dn,i) stride 128-els, col stride 7168, b 1.
            q_written = [0] * NQ

            def stage_write(h1re, h1im, npair, pr):
                nonlocal dma_flip
                for u in range(npair):
                    s0 = (pr + u) * CHI_SITES
                    # rectangles (quarter, col0, ncol, dn0, ndn) not crossing
                    # a quarter boundary nor S_LOC
                    pieces = []
                    for colq in range(CHI_COLS):
                        base = s0 + colq * DN
                        ns = min(DN, max(0, S_LOC - base))
                        st = 0
                        while st < ns:
                            s = base + st
                            qh = s // SQ
                            n_ = min(ns - st, (qh + 1) * SQ - s)
                            pieces.append((qh, colq, st, n_))
                            st += n_
                    rects = []
                    ii = 0
                    while ii < len(pieces):
                        qh, colq, st, n_ = pieces[ii]
                        if st == 0 and n_ == DN:
                            jj = ii
                            while (jj + 1 < len(pieces)
                                   and pieces[jj + 1][:1] == (qh,)
                                   and pieces[jj + 1][2] == 0
                                   and pieces[jj + 1][3] == DN
                                   and pieces[jj + 1][1] == pieces[jj][1] + 1):
                                jj += 1
                            rects.append((qh, colq, pieces[jj][1] - colq + 1,
                                          0, DN))
                            ii = jj + 1
                        else:
                            rects.append((qh, colq, 1, st, n_))
                            ii += 1
                    for (qh, c0, ncol, dn0, ndn) in rects:
                        # group index m = local_site*4 + i; multi-col rects
                        # are always full cols (dn0=0, ndn=DN)
                        m0 = (s0 - qh * SQ + c0 * DN + dn0) * 4
                        for ri, tl in ((0, h1re), (1, h1im)):
                            for gg in range(NG):
                                M = d_stq[qh][gg * NP].rearrange(
                                    "s (m x) -> (s m) x", m=4)
                                if ncol == 1:
                                    dst = M[m0:m0 + 4 * ndn,
                                            ri * 64:(ri + 1) * 64]
                                else:
                                    dst = M[m0:m0 + ncol * 4 * DN, :] \
                                        .rearrange("(c m) x -> m c x",
                                                   c=ncol)[
                                        :, :, ri * 64:(ri + 1) * 64]
                                src = tl[u * 64 + dn0 * 4:
                                         u * 64 + (dn0 + ndn) * 4,
                                         c0:c0 + ncol, gg * 64:(gg + 1) * 64]
                                if ncol == 1:
                                    src = src.rearrange("p c b -> p (c b)")
                                nc.sync.dma_start(dst, src)
                        q_written[qh] += ncol * ndn

            dups_done = 0
            dma_flip = True
            for base in range(0, CHI_NCH, 4):
                nch = min(4, CHI_NCH - base)
                pss = []
                for u in range(nch):
                    ch = base + u
                    gch = pool.tile([128, CHI_COLS, 128], F32, name="gchi",
                                    tag="gchi", bufs=5)
                    nc.gpsimd.dma_gather(
                        gch[:], d_xf[:],
                        t_cgi[:, ch * CHI_COLS * 8:(ch + 1) * CHI_COLS * 8],
                        CHI_COLS * 128, CHI_COLS * 128, 128, elem_step=128)
                    pch = ppsum.tile([128, 512], F32, name="pchi", tag="pchi",
                                     bufs=4)
                    nc.tensor.matmul(
                        pch[:], lhsT=t_wchi[:],
                        rhs=gch[:].rearrange("p a b -> p (a b)"),
                        start=True, stop=True)
                    pss.append(pch)
                h1re = pool.tile([120, 2, CHI_COLS, 128], F16, name="h1re",
                                 tag="h1re", bufs=3)
                h1im = pool.tile([120, 2, CHI_COLS, 128], F16, name="h1im",
                                 tag="h1im", bufs=3)
                emit_ctanh(
                    nc, pool,
                    [(p_[0:56, :], p_[64:120, :], u % 2, u // 2)
                     for u, p_ in enumerate(pss)],
                    h1re[:].rearrange("p g a b -> p (g a b)"),
                    h1im[:].rearrange("p g a b -> p (g a b)"), "ctchi",
                    FG=2)
                for pg in range(2):
                    npair = min(2, nch - 2 * pg)
                    if npair <= 0:
                        break
                    stage_write(h1re[:, pg], h1im[:, pg], npair,
                                base + 2 * pg)
                # fire dup copies + a2a per completed site-half
                while dups_done < NQ and q_written[dups_done] >= SQ:
                    qd = dups_done
                    for gg in range(NG):
                        for pp_ in range(1, NP):
                            eng = nc.sync if dma_flip else nc.scalar
                            dma_flip = not dma_flip
                            eng.dma_start(
                                d_stq[qd][gg * NP + pp_], d_stq[qd][gg * NP])
                    nc.gpsimd.collective_compute(
                        "AllToAll", ALU.bypass,
                        replica_groups=[list(range(NCORES))],
                        ins=[d_stq[qd][:]], outs=[d_recv[qd]])
                    dups_done += 1

            # =========== wilson (quarter p, my 64 batches) ===========
            recv_flat = d_recv[:].rearrange("a b c f -> (a b c) f")
            for ch in range(WIL_NCH):
                gw = pool.tile([128, 4, 2, 512], F16, name="gwil", tag="gwil",
                               bufs=3)
                nc.gpsimd.dma_gather(
                    gw[:].rearrange("p a b f -> p (a b) f"), recv_flat,
                    t_wgi[:, ch * 64:(ch + 1) * 64], 4 * WIL_PC, 4 * WIL_PC,
                    512, elem_step=512)
                m1 = pool.tile([128, 2, 512], F16, name="wm1", tag="wm1",
                               bufs=2)
                m2 = pool.tile([128, 2, 512], F16, name="wm2", tag="wm2",
                               bufs=2)
                h2t = pool.tile([128, 2, 512], F16, name="h2t", tag="h2t",
                                bufs=2)
                nc.vector.tensor_mul(m1[:], gw[:, 0, :, :], gw[:, 1, :, :])
                nc.vector.tensor_mul(m2[:], gw[:, 2, :, :], gw[:, 3, :, :])
                # content reorder [i,ri,b] -> [b,ri,i] for omega's
                # K-slices; DVE APs are limited to 3D, so split by (cc, ri)
                for cc in range(2):
                    for ri in range(2):
                        dstv = h2t[:, cc, :].rearrange(
                            "p (b r i) -> r p b i", b=64, r=2, i=4)[ri]
                        nc.vector.scalar_tensor_tensor(
                            out=dstv,
                            in0=m1[:, cc, :].rearrange(
                                "p (i r b) -> r p b i", i=4, r=2, b=64)[ri],
                            scalar=float(WILSON_RESCALE),
                            in1=m2[:, cc, :].rearrange(
                                "p (i r b) -> r p b i", i=4, r=2, b=64)[ri],
                            op0=ALU.mult, op1=ALU.mult)
                jq, r0 = ch // 4, (ch % 4) * WIL_PC
                nc.sync.dma_start(
                    d_h2q[jq][r0:r0 + WIL_PC, :]
                    .rearrange("(c q) f -> q c f", c=2), h2t[:])
                if ch % 4 == 3:
                    nc.gpsimd.collective_compute(
                        "AllGather", ALU.bypass,
                        replica_groups=[[0, 1, 2, 3], [4, 5, 6, 7]],
                        ins=[d_h2q[jq][:]], outs=[d_h2g[jq]])

            # =========== omega (quarter p, my 64 batches) ===========
            h2g_flat = d_h2g[:].rearrange("a b c f -> (a b c) f")
            acc_re = pool.tile([64, 4], F32, name="acc_re")
            acc_im = pool.tile([64, 4], F32, name="acc_im")
            nc.vector.memset(acc_re[:], 0.0)
            nc.vector.memset(acc_im[:], 0.0)

            def omega_tail(zcs, grp):
                # zcs[j]: bh = 2*grp + j//2, half = j%2; pair (pr_, pr_+2)
                # stacks bh 2*grp (rows 0:64) with bh 2*grp+1 (rows 64:128)
                for pr_ in range(2):
                    za, zb = zcs[pr_], zcs[pr_ + 2]
                    tr = pool.tile([128, 512], F32, name="otr", tag="otr",
                                   bufs=2)
                    ti = pool.tile([128, 512], F32, name="oti", tag="oti",
                                   bufs=2)
                    emit_taylor_ctanh(nc, pool, za, zb, tr[:], ti[:])
                    rr = pool.tile([128, 1], F32, name="orr", tag="orr",
                                   bufs=2)
                    rim = pool.tile([128, 1], F32, name="ori", tag="ori",
                                    bufs=2)
                    nc.vector.tensor_reduce(
                        out=rr[:], in_=tr[:], axis=mybir.AxisListType.X,
                        op=ALU.add)
                    nc.vector.tensor_reduce(
                        out=rim[:], in_=ti[:], axis=mybir.AxisListType.X,
                        op=ALU.add)
                    bha, bhb = 2 * grp, 2 * grp + 1
                    nc.vector.tensor_add(
                        acc_re[:, bha:bha + 1], acc_re[:, bha:bha + 1],
                        rr[0:64, :])
                    nc.vector.tensor_add(
                        acc_im[:, bha:bha + 1], acc_im[:, bha:bha + 1],
                        rim[0:64, :])
                    r2 = pool.tile([64, 1], F32, name="or2", tag="or2",
                                   bufs=2)
                    i2 = pool.tile([64, 1], F32, name="oi2", tag="oi2",
                                   bufs=2)
                    nc.vector.tensor_copy(out=r2[:], in_=rr[64:128, :])
                    nc.vector.tensor_copy(out=i2[:], in_=rim[64:128, :])
                    nc.vector.tensor_add(
                        acc_re[:, bhb:bhb + 1], acc_re[:, bhb:bhb + 1],
                        r2[:])
                    nc.vector.tensor_add(
                        acc_im[:, bhb:bhb + 1], acc_im[:, bhb:bhb + 1],
                        i2[:])

            for ch in range(OMG_NCH):
                buf = ch
                for k in range(K_OMG):
                    gk = pool.tile([128, OMG_PC // 128, 512], F16,
                                   name="gomg", tag="gomg", bufs=5)
                    off = (ch * K_OMG + k) * (OMG_PC // 16)
                    nc.gpsimd.dma_gather(
                        gk[:], h2g_flat, t_ogi[:, off:off + OMG_PC // 16],
                        OMG_PC, OMG_PC, 512, elem_step=512)
                    nc.scalar.dma_start(
                        d_gsc[buf, k].rearrange("(c q) f -> q c f",
                                                c=OMG_PC // 128),
                        gk[:])
                for grp in range(2):
                    zcs = [ppsum.tile([128, 512], F32, name=f"zc{jj}",
                                      tag=f"zc{jj}", bufs=1)
                           for jj in range(4)]
                    for k in range(K_OMG):
                        gT = pool.tile([128, 2, OMG_PC], F16, name="gT",
                                       tag="gT", bufs=5)
                        for bi in range(2):
                            bh = 2 * grp + bi
                            nc.sync.dma_start_transpose(
                                gT[:, bi, :],
                                d_gsc[buf, k][:, 128 * bh:128 * (bh + 1)])
                        for bi in range(2):
                            for hh in range(2):
                                nc.tensor.matmul(
                                    zcs[bi * 2 + hh][:],
                                    lhsT=t_womg[:, k * 128:(k + 1) * 128],
                                    rhs=gT[:, bi, 512 * hh:512 * (hh + 1)],
                                    start=(k == 0), stop=(k == K_OMG - 1))
                    omega_tail(zcs, grp)

            if DEBUG_DUMP:
                nc.sync.dma_start(
                    d_recv_d[:],
                    d_recv[:].rearrange("a b c f -> (a b c) f"))
                nc.sync.dma_start(
                    d_h2g_d[:],
                    d_h2g[:].rearrange("a b c f -> (a b c) f"))
            out_t = pool.tile([64, 8], F32, name="out_t")
            nc.vector.tensor_copy(out=out_t[:, 0:4], in_=acc_re[:])
            nc.vector.tensor_copy(out=out_t[:, 4:8], in_=acc_im[:])
            nc.sync.dma_start(d_out[:], out_t[:])
    nc.compile()
    return nc


_NC_CACHE = None


def kernel(x, chi_kernel_idx, chi_kernel_mask, plaquette_idx, plaquette_mask,
           omega_kernel_idx, omega_kernel_mask, chi_w, chi_b, omega_w,
           omega_b, _want_trace=False):
    global _NC_CACHE
    x = np.asarray(x, np.float32)
    chi_kernel_idx = np.asarray(chi_kernel_idx).astype(np.int64)
    plaquette_idx = np.asarray(plaquette_idx).astype(np.int64)
    omega_kernel_idx = np.asarray(omega_kernel_idx).astype(np.int64)
    chi_w = np.asarray(chi_w)
    omega_w = np.asarray(omega_w)

    chi_gidx_cores, wil_gidx_p, omg_gidx_p, wchi, womg = build_host_tables(
        chi_kernel_idx, plaquette_idx, omega_kernel_idx, chi_w, omega_w)

    # omega uses a cubic Taylor tanh; verify |z_omega| is in-range
    _h1 = np.tanh(np.einsum(
        "bnk,ik->bni", x.astype(np.complex64)[:, chi_kernel_idx],
        chi_w[:, 0, :]))
    _gp = _h1[:, plaquette_idx, :]
    _h2 = (WILSON_RESCALE * _gp.real.prod(axis=2)
           + 1j * WILSON_RESCALE * _gp.imag.prod(axis=2))
    _z = np.einsum("bnki,oik->bno", _h2[:, omega_kernel_idx, :], omega_w)
    _zmax = max(np.abs(_z.real).max(), np.abs(_z.imag).max())
    assert _zmax < 0.25, f"omega pre-tanh out of Taylor range: {_zmax}"

    if _NC_CACHE is None:
        _NC_CACHE = build_kernel()
    nc = _NC_CACHE

    xf = np.zeros((N_SITES + 1, 128), np.float32)
    xf[:N_SITES] = x.T
    in_maps = []
    for c in range(NCORES):
        p = c % NP
        in_maps.append({
            "xf": xf, "cgi": chi_gidx_cores[c], "wgi": wil_gidx_p[p],
            "ogi": omg_gidx_p[p], "wchi": wchi, "womg": womg,
        })
    r = run_bass_kernel_spmd(nc, in_maps, core_ids=list(range(NCORES)),
                             trace=_want_trace)
    out = np.zeros(B, np.complex64)
    for c in range(NCORES):
        g, p = c // NP, c % NP
        o = r.results[c]["out"]
        v = (o[:, 0:4] + 1j * o[:, 4:8]).reshape(16, 4, 4)  # [b_lo, o, bh]
        vv = v.sum(axis=1)                                  # [b_lo, bh]
        for bh in range(4):
            out[g * 64 + bh * 16:g * 64 + bh * 16 + 16] += vv[:, bh]
    kernel._LAST_R = r
    if _want_trace:
        kernel._last_result = r
    return out


# revision 37
# speedup vs baseline: 1.1623x; 1.0274x over previous
"""Trainium2 Bass kernel for nn_ApproxSymmetricNet (gnn_message_passing).

8 NeuronCores, hybrid sharding: core c = (g=c//4 batch-half, p=c%4
plaquette-quarter).

chi is SITE-sharded by 8 (2048 sites x all 128 batches per core; 9 gather
descriptors per site). wilson/omega are (batch-half x plaquette-quarter)
sharded: rows hold 64 batches -> 512 fp16 = 1KB elements, so gather
descriptor counts drop to 4x4096 and 5x4096 per core (the SWDGE gather
ucode costs ~571ns + 7.86ns/descriptor of serial Pool time, independent
of element size -- the kernel's governing cost).

Dataflow: chi (gather x rows -> PE conv -> complex tanh -> PE transpose to
site-major fp16 rows [b64, ri, i]) -> dup-staged AllToAll (each dest quad
gets its batch-half) -> wilson (gather h1 rows, DVE products) -> quad
AllGather -> omega (gather h2 rows -> DRAM scratch -> HWDGE DMA-transpose
puts comps on partitions (PE transpose of gathered volume would be
PE-bound; transpose-mode dma_gather crashes this runtime) -> 5 f16
block-diag matmuls (K=128=(b_lo,ri,i) -> M=(ri_o,b_lo,o)) -> complex tanh
(reciprocal on ACT) -> free-dim reduce). Host sums channels + quarters.

tanh(x+iy) = (2T + i*(1-T^2)*sin2y) / D, D = 2*(1 - (1-T^2)*sin^2(y)).
"""
import numpy as np

import concourse.bacc as bacc
import concourse.mybir as mybir
import concourse.tile as tile
from concourse.bass_utils import run_bass_kernel_spmd

AFT = mybir.ActivationFunctionType
ALU = mybir.AluOpType
F32 = mybir.dt.float32
F16 = mybir.dt.float16
I16 = mybir.dt.int16

B, N_SITES, N_PLAQ = 128, 16384, 16384
K_CHI, P_SZ, K_OMG = 9, 4, 5
C_CHI, C_OMG = 4, 4
WILSON_RESCALE = 10 ** 1.5
NCORES = 8
NG, NP = 2, 4               # batch groups x plaquette quarters
BG = B // NG                # 64 batches per group
S_LOC = N_SITES // NCORES   # 2048 sites per core (chi)
P_LOC = N_PLAQ // NP        # 4096 plaquettes per core (wilson/omega)

# chi chunking (site-sharded)
DN = 14                     # sites per partition-group (14*9=126 partitions)
CHI_COLS = 4
CHI_SITES = DN * CHI_COLS   # 56 sites per chunk
CHI_NCH = (S_LOC + CHI_SITES - 1) // CHI_SITES      # 37

NQ = 2                      # a2a site-halves
SQ = S_LOC // NQ
WIL_PC = 256                # wilson plaquettes per chunk (4*256=1024 idx)
WIL_NCH = P_LOC // WIL_PC   # 16
OMG_PC = 1024               # omega plaquettes per chunk (5 calls of 1024)
OMG_NCH = P_LOC // OMG_PC   # 4
AG_PC = 1024                # h2 AllGather granularity (4 chunks)
DEBUG_DUMP = False


def _wrap_idx16(flat):
    n = len(flat)
    a = flat.reshape(n // 16, 16).T
    return np.tile(a, (8, 1)).astype(np.int16)


def _h1row(n):
    # site n -> row in d_recv [2 q][8 src][1024]
    return (n % S_LOC) // SQ * (NCORES * SQ) + (n // S_LOC) * SQ + n % SQ


def _h2row(m):
    # plaquette m -> row in d_h2g [4j][4rank][1024]
    return (m % P_LOC) // AG_PC * (NP * AG_PC) + (m // P_LOC) * AG_PC \
        + m % AG_PC


def build_host_tables(chi_kernel_idx, plaquette_idx, omega_kernel_idx,
                      chi_w, omega_w):
    # ---- per-core chi gather tables (site shard c) ----
    ci = np.concatenate(
        [chi_kernel_idx, np.full((CHI_SITES, K_CHI), N_SITES, np.int64)])
    chi_gidx_cores = []
    j = np.arange(CHI_NCH * CHI_COLS * 128)
    col = j // 128
    pp = j % 128
    dn = pp // K_CHI
    k = pp % K_CHI
    nl = (j // (CHI_COLS * 128)) * CHI_SITES + col % CHI_COLS * DN \
        + np.minimum(dn, DN - 1)
    for cc in range(NCORES):
        n = np.where(nl < S_LOC, cc * S_LOC + nl, N_SITES)
        flat = ci[np.minimum(n, N_SITES), k]
        flat[pp >= DN * K_CHI] = 0
        chi_gidx_cores.append(_wrap_idx16(flat))

    # ---- per-quarter wilson gather tables ----
    wil_gidx_p = []
    for p in range(NP):
        flat = np.zeros(WIL_NCH * 4 * WIL_PC, np.int64)
        for ch in range(WIL_NCH):
            jj = np.arange(4 * WIL_PC)
            kk = jj // WIL_PC
            cc_ = (jj % WIL_PC) // 128
            qq = jj % 128
            m = p * P_LOC + ch * WIL_PC + cc_ * 128 + qq
            flat[ch * 4 * WIL_PC + jj] = _h1row(plaquette_idx[m, kk])
        wil_gidx_p.append(_wrap_idx16(flat))

    # ---- per-quarter omega gather tables ----
    omg_gidx_p = []
    for p in range(NP):
        flat = np.zeros(OMG_NCH * K_OMG * OMG_PC, np.int64)
        pos = 0
        for ch in range(OMG_NCH):
            for k in range(K_OMG):
                m = p * P_LOC + ch * OMG_PC + np.arange(OMG_PC)
                flat[pos:pos + OMG_PC] = _h2row(omega_kernel_idx[m, k])
                pos += OMG_PC
        omg_gidx_p.append(_wrap_idx16(flat))

    # ---- chi weight lhsT [128,128]: row (dn*9+k) -> col (ri*64+dn*4+i) ----
    wchi = np.zeros((128, 128), np.float32)
    for dn_ in range(DN):
        for k in range(K_CHI):
            for i in range(C_CHI):
                wchi[dn_ * K_CHI + k, 0 * 64 + dn_ * 4 + i] = \
                    chi_w[i, 0, k].real
                wchi[dn_ * K_CHI + k, 1 * 64 + dn_ * 4 + i] = \
                    chi_w[i, 0, k].imag

    # ---- omega weights [128, 5*128] f16:
    # row (b_lo*8 + ri*4 + i) -> col (ri_o*64 + b_lo*4 + o) ----
    woms = []
    for k in range(K_OMG):
        w = np.zeros((128, 128), np.float32)
        for bl in range(16):
            for i in range(C_CHI):
                for o in range(C_OMG):
                    wr = omega_w[o, i, k].real
                    wi = omega_w[o, i, k].imag
                    w[bl * 8 + 0 * 4 + i, 0 * 64 + bl * 4 + o] = wr
                    w[bl * 8 + 1 * 4 + i, 0 * 64 + bl * 4 + o] = -wi
                    w[bl * 8 + 0 * 4 + i, 1 * 64 + bl * 4 + o] = wi
                    w[bl * 8 + 1 * 4 + i, 1 * 64 + bl * 4 + o] = wr
        woms.append(w)
    womg = np.concatenate(woms, axis=1).astype(np.float16)
    return chi_gidx_cores, wil_gidx_p, omg_gidx_p, wchi, womg


def emit_ctanh(nc, pool, pslist, out_re, out_im, tagpfx, FG=1):
    """Complex tanh of psum (re, im) pairs stacked in partitions AND free.

    pslist entries: (sx, sy, u, fg): psum [H, F0] placed at partitions
    [u*64, u*64+H), free [fg*F0, (fg+1)*F0). out_re/out_im [P, FG*F0].
    ACT ops grouped by function to limit table reloads; DVE f16 +
    reciprocal.
    """
    H = pslist[0][0].shape[0]
    P = out_re.shape[0]
    F = out_re.free_size()
    F0 = F // FG

    def t(nm):
        return pool.tile([P, F], F16, name=f"{tagpfx}_{nm}", tag=f"ct_{nm}",
                         bufs=2)
    T_, s_, c_, t2, q_, u_, d_, r_ = (t(x) for x in
                                      ("T", "s", "c", "t2", "q", "u", "d",
                                       "r"))

    def sl2(u, fg):
        return (slice(u * 64, u * 64 + H),
                slice(fg * F0, (fg + 1) * F0))
    for (sx, sy, u, fg) in pslist:
        nc.scalar.activation(T_[sl2(u, fg)], sx, AFT.Tanh)
    for (sx, sy, u, fg) in pslist:
        nc.scalar.activation(s_[sl2(u, fg)], sy, AFT.Sin, scale=2.0)
    for (sx, sy, u, fg) in pslist:
        nc.scalar.activation(c_[sl2(u, fg)], sy, AFT.Sin)
    nc.vector.tensor_mul(t2[:], T_[:], T_[:])
    nc.vector.tensor_scalar(out=q_[:], in0=t2[:], scalar1=-1.0, scalar2=1.0,
                            op0=ALU.mult, op1=ALU.add)
    nc.vector.tensor_mul(u_[:], c_[:], c_[:])
    nc.vector.tensor_mul(u_[:], u_[:], q_[:])
    nc.vector.tensor_scalar(out=d_[:], in0=u_[:], scalar1=-2.0, scalar2=2.0,
                            op0=ALU.mult, op1=ALU.add)
    nc.vector.reciprocal(r_[:], d_[:])
    nc.vector.scalar_tensor_tensor(out=out_re, in0=T_[:], scalar=2.0,
                                   in1=r_[:], op0=ALU.mult, op1=ALU.mult)
    nc.vector.tensor_mul(s_[:], s_[:], q_[:])
    nc.vector.tensor_mul(out_im, s_[:], r_[:])


def emit_taylor_ctanh(nc, pool, za, zb, out_re, out_im):
    """tanh(w) ~= w - w^3/3 = w*(1 - w^2/3) for |w| <~ 0.2 (omega's z is
    <~ 0.05 for this model scale; validated host-side). DVE-only: frees the
    ACT engine and psum banks quickly. za/zb: psum [128,512] with re in
    partitions [0:64], im in [64:128]; stacked as two units."""
    def t(nm):
        return pool.tile([128, 512], F16, name=f"tl_{nm}", tag=f"tl_{nm}",
                         bufs=2)
    zx, zy, x2, y2, q2, p_, pr, t1, v1, v3, v4 = (
        t(x) for x in ("zx", "zy", "x2", "y2", "q2", "p", "pr", "t1", "v1",
                       "v3", "v4"))
    nc.vector.tensor_copy(out=zx[0:64, :], in_=za[0:64, :])
    nc.vector.tensor_copy(out=zx[64:128, :], in_=zb[0:64, :])
    nc.vector.tensor_copy(out=zy[0:64, :], in_=za[64:128, :])
    nc.vector.tensor_copy(out=zy[64:128, :], in_=zb[64:128, :])
    nc.vector.tensor_mul(x2[:], zx[:], zx[:])
    nc.vector.tensor_mul(y2[:], zy[:], zy[:])
    nc.vector.tensor_mul(q2[:], zx[:], zy[:])
    nc.vector.scalar_tensor_tensor(out=p_[:], in0=y2[:], scalar=-1.0,
                                   in1=x2[:], op0=ALU.mult, op1=ALU.add)
    nc.vector.tensor_scalar(out=pr[:], in0=p_[:], scalar1=-1.0 / 3.0,
                            scalar2=1.0, op0=ALU.mult, op1=ALU.add)
    nc.vector.tensor_mul(t1[:], zx[:], pr[:])
    nc.vector.tensor_mul(v1[:], zy[:], q2[:])
    nc.vector.scalar_tensor_tensor(out=out_re, in0=v1[:], scalar=2.0 / 3.0,
                                   in1=t1[:], op0=ALU.mult, op1=ALU.add)
    nc.vector.tensor_mul(v3[:], zx[:], q2[:])
    nc.vector.tensor_mul(v4[:], zy[:], pr[:])
    nc.vector.scalar_tensor_tensor(out=out_im, in0=v3[:], scalar=-2.0 / 3.0,
                                   in1=v4[:], op0=ALU.mult, op1=ALU.add)


def build_kernel():
    nc = bacc.Bacc("TRN2", target_bir_lowering=False, debug=True)

    d_xf = nc.dram_tensor("xf", [N_SITES + 1, 128], F32, kind="ExternalInput")
    d_cgi = nc.dram_tensor("cgi", [128, CHI_NCH * CHI_COLS * 8], I16,
                           kind="ExternalInput")
    d_wgi = nc.dram_tensor("wgi", [128, WIL_NCH * 64], I16,
                           kind="ExternalInput")
    d_ogi = nc.dram_tensor("ogi", [128, OMG_NCH * K_OMG * (OMG_PC // 16)], I16,
                           kind="ExternalInput")
    d_wchi = nc.dram_tensor("wchi", [128, 128], F32, kind="ExternalInput")
    d_womg = nc.dram_tensor("womg", [128, K_OMG * 128], F16,
                            kind="ExternalInput")
    # a2a staging: per site-half, 8 dest blocks of [1024 sites, 512 f16]
    d_stq = [nc.dram_tensor(f"stq{q}", [NCORES, SQ, 512], F16)
             for q in range(NQ)]
    # h1 for my batch group: rows q*8192 + src*1024 + loc
    d_recv = nc.dram_tensor("recv", [NQ, NCORES, SQ, 512], F16)
    # wilson output shard, j-major quarters of my 4096 plaquettes
    d_h2q = [nc.dram_tensor(f"h2q{j}", [AG_PC, 512], F16)
             for j in range(4)]
    # h2 full for my batch group: rows j*4096 + rank*1024 + loc
    d_h2g = nc.dram_tensor("h2g", [4, NP, AG_PC, 512], F16)
    if DEBUG_DUMP:
        d_recv_d = nc.dram_tensor("recv_d", [NQ * NCORES * SQ, 512], F16,
                                  kind="ExternalOutput")
        d_h2g_d = nc.dram_tensor("h2g_d", [4 * NP * AG_PC, 512], F16,
                                 kind="ExternalOutput")
        d_gt_d = nc.dram_tensor("gt_d", [128, OMG_PC], F16,
                                kind="ExternalOutput")
        d_zc_d = nc.dram_tensor("zc_d", [128, 512], F32,
                                kind="ExternalOutput")
        d_tr_d = nc.dram_tensor("tr_d", [128, 1024], F32,
                                kind="ExternalOutput")
    # omega gather scratch (double-buffered over chunks)
    d_gsc = nc.dram_tensor("gsc", [OMG_NCH, K_OMG, OMG_PC, 512], F16)
    d_out = nc.dram_tensor("out", [64, 8], F32, kind="ExternalOutput")

    with tile.TileContext(nc) as tc, \
         nc.allow_low_precision(reason="fp16 rows/tanh; tol 2e-2"):
        with tc.tile_pool(name="pidx", bufs=1) as pidx, \
             tc.tile_pool(name="pwork", bufs=1) as pool, \
             tc.tile_pool(name="ppsum", bufs=1, space="PSUM") as ppsum:
            t_cgi = pidx.tile([128, CHI_NCH * CHI_COLS * 8], I16,
                              name="t_cgi")
            t_wgi = pidx.tile([128, WIL_NCH * 64], I16, name="t_wgi")
            t_ogi = pidx.tile([128, OMG_NCH * K_OMG * (OMG_PC // 16)], I16,
                              name="t_ogi")
            t_wchi = pidx.tile([128, 128], F32, name="t_wchi")
            t_womg = pidx.tile([128, K_OMG * 128], F16, name="t_womg")
            nc.sync.dma_start(t_cgi[:], d_cgi[:])
            nc.sync.dma_start(t_wgi[:], d_wgi[:])
            nc.sync.dma_start(t_ogi[:], d_ogi[:])
            nc.sync.dma_start(t_wchi[:], d_wchi[:])
            nc.sync.dma_start(t_womg[:], d_womg[:])

            # =========== chi (site-sharded, all 128 batches) ===========
            # h1 row content is [i(4), ri(2), b(64)]: the stage write from
            # ctanh's [(dn,i) parts, (col,b)] layout is then a 3-dim AP with
            # 128B runs: merged (dn,i) stride 128-els, col stride 7168, b 1.
            q_written = [0] * NQ

            def stage_write(h1re, h1im, npair, pr):
                nonlocal dma_flip
                for u in range(npair):
                    s0 = (pr + u) * CHI_SITES
                    # rectangles (quarter, col0, ncol, dn0, ndn) not crossing
                    # a quarter boundary nor S_LOC
                    pieces = []
                    for colq in range(CHI_COLS):
                        base = s0 + colq * DN
                        ns = min(DN, max(0, S_LOC - base))
                        st = 0
                        while st < ns:
                            s = base + st
                            qh = s // SQ
                            n_ = min(ns - st, (qh + 1) * SQ - s)
                            pieces.append((qh, colq, st, n_))
                            st += n_
                    rects = []
                    ii = 0
                    while ii < len(pieces):
                        qh, colq, st, n_ = pieces[ii]
                        if st == 0 and n_ == DN:
                            jj = ii
                            while (jj + 1 < len(pieces)
                                   and pieces[jj + 1][:1] == (qh,)
                                   and pieces[jj + 1][2] == 0
                                   and pieces[jj + 1][3] == DN
                                   and pieces[jj + 1][1] == pieces[jj][1] + 1):
                                jj += 1
                            rects.append((qh, colq, pieces[jj][1] - colq + 1,
                                          0, DN))
                            ii = jj + 1
                        else:
                            rects.append((qh, colq, 1, st, n_))
                            ii += 1
                    for (qh, c0, ncol, dn0, ndn) in rects:
                        # group index m = local_site*4 + i; multi-col rects
                        # are always full cols (dn0=0, ndn=DN)
                        m0 = (s0 - qh * SQ + c0 * DN + dn0) * 4
                        for ri, tl in ((0, h1re), (1, h1im)):
                            for gg in range(NG):
                                M = d_stq[qh][gg * NP].rearrange(
                                    "s (m x) -> (s m) x", m=4)
                                if ncol == 1:
                                    dst = M[m0:m0 + 4 * ndn,
                                            ri * 64:(ri + 1) * 64]
                                else:
                                    dst = M[m0:m0 + ncol * 4 * DN, :] \
                                        .rearrange("(c m) x -> m c x",
                                                   c=ncol)[
                                        :, :, ri * 64:(ri + 1) * 64]
                                src = tl[u * 64 + dn0 * 4:
                                         u * 64 + (dn0 + ndn) * 4,
                                         c0:c0 + ncol, gg * 64:(gg + 1) * 64]
                                if ncol == 1:
                                    src = src.rearrange("p c b -> p (c b)")
                                nc.sync.dma_start(dst, src)
                        q_written[qh] += ncol * ndn

            dups_done = 0
            dup_rows = 0
            dma_flip = True
            for base in range(0, CHI_NCH, 4):
                nch = min(4, CHI_NCH - base)
                pss = []
                for u in range(nch):
                    ch = base + u
                    gch = pool.tile([128, CHI_COLS, 128], F32, name="gchi",
                                    tag="gchi", bufs=5)
                    nc.gpsimd.dma_gather(
                        gch[:], d_xf[:],
                        t_cgi[:, ch * CHI_COLS * 8:(ch + 1) * CHI_COLS * 8],
                        CHI_COLS * 128, CHI_COLS * 128, 128, elem_step=128)
                    pch = ppsum.tile([128, 512], F32, name="pchi", tag="pchi",
                                     bufs=4)
                    nc.tensor.matmul(
                        pch[:], lhsT=t_wchi[:],
                        rhs=gch[:].rearrange("p a b -> p (a b)"),
                        start=True, stop=True)
                    pss.append(pch)
                h1re = pool.tile([120, 2, CHI_COLS, 128], F16, name="h1re",
                                 tag="h1re", bufs=3)
                h1im = pool.tile([120, 2, CHI_COLS, 128], F16, name="h1im",
                                 tag="h1im", bufs=3)
                emit_ctanh(
                    nc, pool,
                    [(p_[0:56, :], p_[64:120, :], u % 2, u // 2)
                     for u, p_ in enumerate(pss)],
                    h1re[:].rearrange("p g a b -> p (g a b)"),
                    h1im[:].rearrange("p g a b -> p (g a b)"), "ctchi",
                    FG=2)
                for pg in range(2):
                    npair = min(2, nch - 2 * pg)
                    if npair <= 0:
                        break
                    stage_write(h1re[:, pg], h1im[:, pg], npair,
                                base + 2 * pg)
                # fire 512-row dup slices as rows complete; a2a per half
                total_written = sum(q_written)
                while dup_rows < total_written // 512 * 512:
                    r0 = dup_rows
                    qd, lr = r0 // SQ, r0 % SQ
                    for gg in range(NG):
                        for pp_ in range(1, NP):
                            eng = nc.sync if dma_flip else nc.scalar
                            dma_flip = not dma_flip
                            eng.dma_start(
                                d_stq[qd][gg * NP + pp_, lr:lr + 512, :],
                                d_stq[qd][gg * NP, lr:lr + 512, :])
                    dup_rows += 512
                while dups_done < NQ and dup_rows >= (dups_done + 1) * SQ:
                    qd = dups_done
                    nc.gpsimd.collective_compute(
                        "AllToAll", ALU.bypass,
                        replica_groups=[list(range(NCORES))],
                        ins=[d_stq[qd][:]], outs=[d_recv[qd]])
                    dups_done += 1

            # =========== wilson (quarter p, my 64 batches) ===========
            recv_flat = d_recv[:].rearrange("a b c f -> (a b c) f")
            for ch in range(WIL_NCH):
                gw = pool.tile([128, 4, 2, 512], F16, name="gwil", tag="gwil",
                               bufs=4)
                nc.gpsimd.dma_gather(
                    gw[:].rearrange("p a b f -> p (a b) f"), recv_flat,
                    t_wgi[:, ch * 64:(ch + 1) * 64], 4 * WIL_PC, 4 * WIL_PC,
                    512, elem_step=512)
                m1 = pool.tile([128, 2, 512], F16, name="wm1", tag="wm1",
                               bufs=2)
                m2 = pool.tile([128, 2, 512], F16, name="wm2", tag="wm2",
                               bufs=2)
                h2t = pool.tile([128, 2, 512], F16, name="h2t", tag="h2t",
                                bufs=2)
                nc.vector.tensor_mul(m1[:], gw[:, 0, :, :], gw[:, 1, :, :])
                nc.vector.tensor_mul(m2[:], gw[:, 2, :, :], gw[:, 3, :, :])
                # content reorder [i,ri,b] -> [b,ri,i] for omega's
                # K-slices; DVE APs are limited to 3D, so split by (cc, ri)
                for cc in range(2):
                    for ri in range(2):
                        dstv = h2t[:, cc, :].rearrange(
                            "p (b r i) -> r p b i", b=64, r=2, i=4)[ri]
                        nc.vector.scalar_tensor_tensor(
                            out=dstv,
                            in0=m1[:, cc, :].rearrange(
                                "p (i r b) -> r p b i", i=4, r=2, b=64)[ri],
                            scalar=float(WILSON_RESCALE),
                            in1=m2[:, cc, :].rearrange(
                                "p (i r b) -> r p b i", i=4, r=2, b=64)[ri],
                            op0=ALU.mult, op1=ALU.mult)
                jq, r0 = ch // 4, (ch % 4) * WIL_PC
                nc.sync.dma_start(
                    d_h2q[jq][r0:r0 + WIL_PC, :]
                    .rearrange("(c q) f -> q c f", c=2), h2t[:])
                if ch % 4 == 3:
                    nc.gpsimd.collective_compute(
                        "AllGather", ALU.bypass,
                        replica_groups=[[0, 1, 2, 3], [4, 5, 6, 7]],
                        ins=[d_h2q[jq][:]], outs=[d_h2g[jq]])

            # =========== omega (quarter p, my 64 batches) ===========
            h2g_flat = d_h2g[:].rearrange("a b c f -> (a b c) f")
            acc_re = pool.tile([64, 4], F32, name="acc_re")
            acc_im = pool.tile([64, 4], F32, name="acc_im")
            nc.vector.memset(acc_re[:], 0.0)
            nc.vector.memset(acc_im[:], 0.0)

            def omega_tail(zcs, grp):
                # zcs[j]: bh = 2*grp + j//2, half = j%2; pair (pr_, pr_+2)
                # stacks bh 2*grp (rows 0:64) with bh 2*grp+1 (rows 64:128)
                for pr_ in range(2):
                    za, zb = zcs[pr_], zcs[pr_ + 2]
                    tr = pool.tile([128, 512], F32, name="otr", tag="otr",
                                   bufs=2)
                    ti = pool.tile([128, 512], F32, name="oti", tag="oti",
                                   bufs=2)
                    emit_taylor_ctanh(nc, pool, za, zb, tr[:], ti[:])
                    rr = pool.tile([128, 1], F32, name="orr", tag="orr",
                                   bufs=2)
                    rim = pool.tile([128, 1], F32, name="ori", tag="ori",
                                    bufs=2)
                    nc.vector.tensor_reduce(
                        out=rr[:], in_=tr[:], axis=mybir.AxisListType.X,
                        op=ALU.add)
                    nc.vector.tensor_reduce(
                        out=rim[:], in_=ti[:], axis=mybir.AxisListType.X,
                        op=ALU.add)
                    bha, bhb = 2 * grp, 2 * grp + 1
                    nc.vector.tensor_add(
                        acc_re[:, bha:bha + 1], acc_re[:, bha:bha + 1],
                        rr[0:64, :])
                    nc.vector.tensor_add(
                        acc_im[:, bha:bha + 1], acc_im[:, bha:bha + 1],
                        rim[0:64, :])
                    r2 = pool.tile([64, 1], F32, name="or2", tag="or2",
                                   bufs=2)
                    i2 = pool.tile([64, 1], F32, name="oi2", tag="oi2",
                                   bufs=2)
                    nc.vector.tensor_copy(out=r2[:], in_=rr[64:128, :])
                    nc.vector.tensor_copy(out=i2[:], in_=rim[64:128, :])
                    nc.vector.tensor_add(
                        acc_re[:, bhb:bhb + 1], acc_re[:, bhb:bhb + 1],
                        r2[:])
                    nc.vector.tensor_add(
                        acc_im[:, bhb:bhb + 1], acc_im[:, bhb:bhb + 1],
                        i2[:])

            for ch in range(OMG_NCH):
                buf = ch
                for k in range(K_OMG):
                    gk = pool.tile([128, OMG_PC // 128, 512], F16,
                                   name="gomg", tag="gomg", bufs=5)
                    off = (ch * K_OMG + k) * (OMG_PC // 16)
                    nc.gpsimd.dma_gather(
                        gk[:], h2g_flat, t_ogi[:, off:off + OMG_PC // 16],
                        OMG_PC, OMG_PC, 512, elem_step=512)
                    nc.scalar.dma_start(
                        d_gsc[buf, k].rearrange("(c q) f -> q c f",
                                                c=OMG_PC // 128),
                        gk[:])
                for grp in range(2):
                    zcs = [ppsum.tile([128, 512], F32, name=f"zc{jj}",
                                      tag=f"zc{jj}", bufs=1)
                           for jj in range(4)]
                    for k in range(K_OMG):
                        gT = pool.tile([128, 2, OMG_PC], F16, name="gT",
                                       tag="gT", bufs=5)
                        for bi in range(2):
                            bh = 2 * grp + bi
                            nc.sync.dma_start_transpose(
                                gT[:, bi, :],
                                d_gsc[buf, k][:, 128 * bh:128 * (bh + 1)])
                        for bi in range(2):
                            for hh in range(2):
                                nc.tensor.matmul(
                                    zcs[bi * 2 + hh][:],
                                    lhsT=t_womg[:, k * 128:(k + 1) * 128],
                                    rhs=gT[:, bi, 512 * hh:512 * (hh + 1)],
                                    start=(k == 0), stop=(k == K_OMG - 1))
                    omega_tail(zcs, grp)

            if DEBUG_DUMP:
                nc.sync.dma_start(
                    d_recv_d[:],
                    d_recv[:].rearrange("a b c f -> (a b c) f"))
                nc.sync.dma_start(
                    d_h2g_d[:],
                    d_h2g[:].rearrange("a b c f -> (a b c) f"))
            out_t = pool.tile([64, 8], F32, name="out_t")
            nc.vector.tensor_copy(out=out_t[:, 0:4], in_=acc_re[:])
            nc.vector.tensor_copy(out=out_t[:, 4:8], in_=acc_im[:])
            nc.sync.dma_start(d_out[:], out_t[:])
    nc.compile()
    return nc


_NC_CACHE = None


def kernel(x, chi_kernel_idx, chi_kernel_mask, plaquette_idx, plaquette_mask,
           omega_kernel_idx, omega_kernel_mask, chi_w, chi_b, omega_w,
           omega_b, _want_trace=False):
    global _NC_CACHE
    x = np.asarray(x, np.float32)
    chi_kernel_idx = np.asarray(chi_kernel_idx).astype(np.int64)
    plaquette_idx = np.asarray(plaquette_idx).astype(np.int64)
    omega_kernel_idx = np.asarray(omega_kernel_idx).astype(np.int64)
    chi_w = np.asarray(chi_w)
    omega_w = np.asarray(omega_w)

    chi_gidx_cores, wil_gidx_p, omg_gidx_p, wchi, womg = build_host_tables(
        chi_kernel_idx, plaquette_idx, omega_kernel_idx, chi_w, omega_w)

    # omega uses a cubic Taylor tanh; verify |z_omega| is in-range
    _h1 = np.tanh(np.einsum(
        "bnk,ik->bni", x.astype(np.complex64)[:, chi_kernel_idx],
        chi_w[:, 0, :]))
    _gp = _h1[:, plaquette_idx, :]
    _h2 = (WILSON_RESCALE * _gp.real.prod(axis=2)
           + 1j * WILSON_RESCALE * _gp.imag.prod(axis=2))
    _z = np.einsum("bnki,oik->bno", _h2[:, omega_kernel_idx, :], omega_w)
    _zmax = max(np.abs(_z.real).max(), np.abs(_z.imag).max())
    assert _zmax < 0.25, f"omega pre-tanh out of Taylor range: {_zmax}"

    if _NC_CACHE is None:
        _NC_CACHE = build_kernel()
    nc = _NC_CACHE

    xf = np.zeros((N_SITES + 1, 128), np.float32)
    xf[:N_SITES] = x.T
    in_maps = []
    for c in range(NCORES):
        p = c % NP
        in_maps.append({
            "xf": xf, "cgi": chi_gidx_cores[c], "wgi": wil_gidx_p[p],
            "ogi": omg_gidx_p[p], "wchi": wchi, "womg": womg,
        })
    r = run_bass_kernel_spmd(nc, in_maps, core_ids=list(range(NCORES)),
                             trace=_want_trace)
    out = np.zeros(B, np.complex64)
    for c in range(NCORES):
        g, p = c // NP, c % NP
        o = r.results[c]["out"]
        v = (o[:, 0:4] + 1j * o[:, 4:8]).reshape(16, 4, 4)  # [b_lo, o, bh]
        vv = v.sum(axis=1)                                  # [b_lo, bh]
        for bh in range(4):
            out[g * 64 + bh * 16:g * 64 + bh * 16 + 16] += vv[:, bh]
    kernel._LAST_R = r
    if _want_trace:
        kernel._last_result = r
    return out


# revision 38
# speedup vs baseline: 1.1689x; 1.0057x over previous
"""Trainium2 Bass kernel for nn_ApproxSymmetricNet (gnn_message_passing).

8 NeuronCores, hybrid sharding: core c = (g=c//4 batch-half, p=c%4
plaquette-quarter).

chi is SITE-sharded by 8 (2048 sites x all 128 batches per core; 9 gather
descriptors per site). wilson/omega are (batch-half x plaquette-quarter)
sharded: rows hold 64 batches -> 512 fp16 = 1KB elements, so gather
descriptor counts drop to 4x4096 and 5x4096 per core (the SWDGE gather
ucode costs ~571ns + 7.86ns/descriptor of serial Pool time, independent
of element size -- the kernel's governing cost).

Dataflow: chi (gather x rows -> PE conv -> complex tanh; h1 rows land
site-major with content [i, ri, b] so the staging write from the
[(dn,i), (col,b)] ctanh layout is a 3-dim AP with 128B runs) ->
dup-staged chunked AllToAll (each dest quad gets its batch-half; dup
copies fire per 512 rows behind chi) -> wilson (gather h1 rows, DVE
products; the output STT reorders row content to [b, ri, i] for omega's
K-slices) -> chunked quad AllGather -> omega (gather h2 rows -> DRAM
scratch -> HWDGE DMA-transpose on the SYNC ring only (the ACT-ring
variant corrupts data; transpose-mode dma_gather crashes the runtime;
2048-row transposes hang the device) -> 5 f16 block-diag matmuls
(K=128=(b_lo,ri,i) -> M=(ri_o,b_lo,o)) -> cubic-Taylor complex tanh
(DVE-only; |z_omega| <~ 0.05, asserted host-side) -> free-dim reduce).
Host sums channels + quarters.

chi tanh(x+iy) = (2T + i*(1-T^2)*sin2y) / D, D = 2*(1-(1-T^2)*sin^2(y)).
"""
import numpy as np

import concourse.bacc as bacc
import concourse.mybir as mybir
import concourse.tile as tile
from concourse.bass_utils import run_bass_kernel_spmd

AFT = mybir.ActivationFunctionType
ALU = mybir.AluOpType
F32 = mybir.dt.float32
F16 = mybir.dt.float16
I16 = mybir.dt.int16

B, N_SITES, N_PLAQ = 128, 16384, 16384
K_CHI, P_SZ, K_OMG = 9, 4, 5
C_CHI, C_OMG = 4, 4
WILSON_RESCALE = 10 ** 1.5
NCORES = 8
NG, NP = 2, 4               # batch groups x plaquette quarters
BG = B // NG                # 64 batches per group
S_LOC = N_SITES // NCORES   # 2048 sites per core (chi)
P_LOC = N_PLAQ // NP        # 4096 plaquettes per core (wilson/omega)

# chi chunking (site-sharded)
DN = 14                     # sites per partition-group (14*9=126 partitions)
CHI_COLS = 4
CHI_SITES = DN * CHI_COLS   # 56 sites per chunk
CHI_NCH = (S_LOC + CHI_SITES - 1) // CHI_SITES      # 37

NQ = 2                      # a2a site-halves
SQ = S_LOC // NQ
WIL_PC = 256                # wilson plaquettes per chunk (4*256=1024 idx)
WIL_NCH = P_LOC // WIL_PC   # 16
OMG_PC = 1024               # omega plaquettes per chunk (5 calls of 1024)
OMG_NCH = P_LOC // OMG_PC   # 4
AG_PC = 1024                # h2 AllGather granularity (4 chunks)
DEBUG_DUMP = False


def _wrap_idx16(flat):
    n = len(flat)
    a = flat.reshape(n // 16, 16).T
    return np.tile(a, (8, 1)).astype(np.int16)


def _h1row(n):
    # site n -> row in d_recv [2 q][8 src][1024]
    return (n % S_LOC) // SQ * (NCORES * SQ) + (n // S_LOC) * SQ + n % SQ


def _h2row(m):
    # plaquette m -> row in d_h2g [4j][4rank][1024]
    return (m % P_LOC) // AG_PC * (NP * AG_PC) + (m // P_LOC) * AG_PC \
        + m % AG_PC


def build_host_tables(chi_kernel_idx, plaquette_idx, omega_kernel_idx,
                      chi_w, omega_w):
    # ---- per-core chi gather tables (site shard c) ----
    ci = np.concatenate(
        [chi_kernel_idx, np.full((CHI_SITES, K_CHI), N_SITES, np.int64)])
    chi_gidx_cores = []
    j = np.arange(CHI_NCH * CHI_COLS * 128)
    col = j // 128
    pp = j % 128
    dn = pp // K_CHI
    k = pp % K_CHI
    nl = (j // (CHI_COLS * 128)) * CHI_SITES + col % CHI_COLS * DN \
        + np.minimum(dn, DN - 1)
    for cc in range(NCORES):
        n = np.where(nl < S_LOC, cc * S_LOC + nl, N_SITES)
        flat = ci[np.minimum(n, N_SITES), k]
        flat[pp >= DN * K_CHI] = 0
        chi_gidx_cores.append(_wrap_idx16(flat))

    # ---- per-quarter wilson gather tables ----
    wil_gidx_p = []
    for p in range(NP):
        flat = np.zeros(WIL_NCH * 4 * WIL_PC, np.int64)
        for ch in range(WIL_NCH):
            jj = np.arange(4 * WIL_PC)
            kk = jj // WIL_PC
            cc_ = (jj % WIL_PC) // 128
            qq = jj % 128
            m = p * P_LOC + ch * WIL_PC + cc_ * 128 + qq
            flat[ch * 4 * WIL_PC + jj] = _h1row(plaquette_idx[m, kk])
        wil_gidx_p.append(_wrap_idx16(flat))

    # ---- per-quarter omega gather tables ----
    omg_gidx_p = []
    for p in range(NP):
        flat = np.zeros(OMG_NCH * K_OMG * OMG_PC, np.int64)
        pos = 0
        for ch in range(OMG_NCH):
            for k in range(K_OMG):
                m = p * P_LOC + ch * OMG_PC + np.arange(OMG_PC)
                flat[pos:pos + OMG_PC] = _h2row(omega_kernel_idx[m, k])
                pos += OMG_PC
        omg_gidx_p.append(_wrap_idx16(flat))

    # ---- chi weight lhsT [128,128]: row (dn*9+k) -> col (ri*64+dn*4+i) ----
    wchi = np.zeros((128, 128), np.float32)
    for dn_ in range(DN):
        for k in range(K_CHI):
            for i in range(C_CHI):
                wchi[dn_ * K_CHI + k, 0 * 64 + dn_ * 4 + i] = \
                    chi_w[i, 0, k].real
                wchi[dn_ * K_CHI + k, 1 * 64 + dn_ * 4 + i] = \
                    chi_w[i, 0, k].imag

    # ---- omega weights [128, 5*128] f16:
    # row (b_lo*8 + ri*4 + i) -> col (ri_o*64 + b_lo*4 + o) ----
    woms = []
    for k in range(K_OMG):
        w = np.zeros((128, 128), np.float32)
        for bl in range(16):
            for i in range(C_CHI):
                for o in range(C_OMG):
                    wr = omega_w[o, i, k].real
                    wi = omega_w[o, i, k].imag
                    w[bl * 8 + 0 * 4 + i, 0 * 64 + bl * 4 + o] = wr
                    w[bl * 8 + 1 * 4 + i, 0 * 64 + bl * 4 + o] = -wi
                    w[bl * 8 + 0 * 4 + i, 1 * 64 + bl * 4 + o] = wi
                    w[bl * 8 + 1 * 4 + i, 1 * 64 + bl * 4 + o] = wr
        woms.append(w)
    womg = np.concatenate(woms, axis=1).astype(np.float16)
    return chi_gidx_cores, wil_gidx_p, omg_gidx_p, wchi, womg


def emit_ctanh(nc, pool, pslist, out_re, out_im, tagpfx, FG=1):
    """Complex tanh of psum (re, im) pairs stacked in partitions AND free.

    pslist entries: (sx, sy, u, fg): psum [H, F0] placed at partitions
    [u*64, u*64+H), free [fg*F0, (fg+1)*F0). out_re/out_im [P, FG*F0].
    ACT ops grouped by function to limit table reloads; DVE f16 +
    reciprocal.
    """
    H = pslist[0][0].shape[0]
    P = out_re.shape[0]
    F = out_re.free_size()
    F0 = F // FG

    def t(nm):
        return pool.tile([P, F], F16, name=f"{tagpfx}_{nm}", tag=f"ct_{nm}",
                         bufs=2)
    T_, s_, c_, t2, q_, u_, d_, r_ = (t(x) for x in
                                      ("T", "s", "c", "t2", "q", "u", "d",
                                       "r"))

    def sl2(u, fg):
        return (slice(u * 64, u * 64 + H),
                slice(fg * F0, (fg + 1) * F0))
    for (sx, sy, u, fg) in pslist:
        nc.scalar.activation(T_[sl2(u, fg)], sx, AFT.Tanh)
    for (sx, sy, u, fg) in pslist:
        nc.scalar.activation(s_[sl2(u, fg)], sy, AFT.Sin, scale=2.0)
    for (sx, sy, u, fg) in pslist:
        nc.scalar.activation(c_[sl2(u, fg)], sy, AFT.Sin)
    nc.vector.tensor_mul(t2[:], T_[:], T_[:])
    nc.vector.tensor_scalar(out=q_[:], in0=t2[:], scalar1=-1.0, scalar2=1.0,
                            op0=ALU.mult, op1=ALU.add)
    nc.vector.tensor_mul(u_[:], c_[:], c_[:])
    nc.vector.tensor_mul(u_[:], u_[:], q_[:])
    nc.vector.tensor_scalar(out=d_[:], in0=u_[:], scalar1=-2.0, scalar2=2.0,
                            op0=ALU.mult, op1=ALU.add)
    nc.vector.reciprocal(r_[:], d_[:])
    nc.vector.scalar_tensor_tensor(out=out_re, in0=T_[:], scalar=2.0,
                                   in1=r_[:], op0=ALU.mult, op1=ALU.mult)
    nc.vector.tensor_mul(s_[:], s_[:], q_[:])
    nc.vector.tensor_mul(out_im, s_[:], r_[:])


def emit_taylor_ctanh(nc, pool, za, zb, out_re, out_im):
    """tanh(w) ~= w - w^3/3 = w*(1 - w^2/3) for |w| <~ 0.2 (omega's z is
    <~ 0.05 for this model scale; validated host-side). DVE-only: frees the
    ACT engine and psum banks quickly. za/zb: psum [128,512] with re in
    partitions [0:64], im in [64:128]; stacked as two units."""
    def t(nm):
        return pool.tile([128, 512], F16, name=f"tl_{nm}", tag=f"tl_{nm}",
                         bufs=2)
    zx, zy, x2, y2, q2, p_, pr, t1, v1, v3, v4 = (
        t(x) for x in ("zx", "zy", "x2", "y2", "q2", "p", "pr", "t1", "v1",
                       "v3", "v4"))
    nc.vector.tensor_copy(out=zx[0:64, :], in_=za[0:64, :])
    nc.vector.tensor_copy(out=zx[64:128, :], in_=zb[0:64, :])
    nc.vector.tensor_copy(out=zy[0:64, :], in_=za[64:128, :])
    nc.vector.tensor_copy(out=zy[64:128, :], in_=zb[64:128, :])
    nc.vector.tensor_mul(x2[:], zx[:], zx[:])
    nc.vector.tensor_mul(y2[:], zy[:], zy[:])
    nc.vector.tensor_mul(q2[:], zx[:], zy[:])
    nc.vector.scalar_tensor_tensor(out=p_[:], in0=y2[:], scalar=-1.0,
                                   in1=x2[:], op0=ALU.mult, op1=ALU.add)
    nc.vector.tensor_scalar(out=pr[:], in0=p_[:], scalar1=-1.0 / 3.0,
                            scalar2=1.0, op0=ALU.mult, op1=ALU.add)
    nc.vector.tensor_mul(t1[:], zx[:], pr[:])
    nc.vector.tensor_mul(v1[:], zy[:], q2[:])
    nc.vector.scalar_tensor_tensor(out=out_re, in0=v1[:], scalar=2.0 / 3.0,
                                   in1=t1[:], op0=ALU.mult, op1=ALU.add)
    nc.vector.tensor_mul(v3[:], zx[:], q2[:])
    nc.vector.tensor_mul(v4[:], zy[:], pr[:])
    nc.vector.scalar_tensor_tensor(out=out_im, in0=v3[:], scalar=-2.0 / 3.0,
                                   in1=v4[:], op0=ALU.mult, op1=ALU.add)


def build_kernel():
    nc = bacc.Bacc("TRN2", target_bir_lowering=False, debug=True)

    d_xf = nc.dram_tensor("xf", [N_SITES + 1, 128], F32, kind="ExternalInput")
    d_cgi = nc.dram_tensor("cgi", [128, CHI_NCH * CHI_COLS * 8], I16,
                           kind="ExternalInput")
    d_wgi = nc.dram_tensor("wgi", [128, WIL_NCH * 64], I16,
                           kind="ExternalInput")
    d_ogi = nc.dram_tensor("ogi", [128, OMG_NCH * K_OMG * (OMG_PC // 16)], I16,
                           kind="ExternalInput")
    d_wchi = nc.dram_tensor("wchi", [128, 128], F32, kind="ExternalInput")
    d_womg = nc.dram_tensor("womg", [128, K_OMG * 128], F16,
                            kind="ExternalInput")
    # a2a staging: per site-half, 8 dest blocks of [1024 sites, 512 f16]
    d_stq = [nc.dram_tensor(f"stq{q}", [NCORES, SQ, 512], F16)
             for q in range(NQ)]
    # h1 for my batch group: rows q*8192 + src*1024 + loc
    d_recv = nc.dram_tensor("recv", [NQ, NCORES, SQ, 512], F16)
    # wilson output shard, j-major quarters of my 4096 plaquettes
    d_h2q = [nc.dram_tensor(f"h2q{j}", [AG_PC, 512], F16)
             for j in range(4)]
    # h2 full for my batch group: rows j*4096 + rank*1024 + loc
    d_h2g = nc.dram_tensor("h2g", [4, NP, AG_PC, 512], F16)
    if DEBUG_DUMP:
        d_recv_d = nc.dram_tensor("recv_d", [NQ * NCORES * SQ, 512], F16,
                                  kind="ExternalOutput")
        d_h2g_d = nc.dram_tensor("h2g_d", [4 * NP * AG_PC, 512], F16,
                                 kind="ExternalOutput")
        d_gt_d = nc.dram_tensor("gt_d", [128, OMG_PC], F16,
                                kind="ExternalOutput")
        d_zc_d = nc.dram_tensor("zc_d", [128, 512], F32,
                                kind="ExternalOutput")
        d_tr_d = nc.dram_tensor("tr_d", [128, 1024], F32,
                                kind="ExternalOutput")
    # omega gather scratch (double-buffered over chunks)
    d_gsc = nc.dram_tensor("gsc", [OMG_NCH, K_OMG, OMG_PC, 512], F16)
    d_out = nc.dram_tensor("out", [64, 8], F32, kind="ExternalOutput")

    with tile.TileContext(nc) as tc, \
         nc.allow_low_precision(reason="fp16 rows/tanh; tol 2e-2"):
        with tc.tile_pool(name="pidx", bufs=1) as pidx, \
             tc.tile_pool(name="pwork", bufs=1) as pool, \
             tc.tile_pool(name="ppsum", bufs=1, space="PSUM") as ppsum:
            t_cgi = pidx.tile([128, CHI_NCH * CHI_COLS * 8], I16,
                              name="t_cgi")
            t_wgi = pidx.tile([128, WIL_NCH * 64], I16, name="t_wgi")
            t_ogi = pidx.tile([128, OMG_NCH * K_OMG * (OMG_PC // 16)], I16,
                              name="t_ogi")
            t_wchi = pidx.tile([128, 128], F32, name="t_wchi")
            t_womg = pidx.tile([128, K_OMG * 128], F16, name="t_womg")
            nc.sync.dma_start(t_cgi[:], d_cgi[:])
            nc.sync.dma_start(t_wgi[:], d_wgi[:])
            nc.sync.dma_start(t_ogi[:], d_ogi[:])
            nc.sync.dma_start(t_wchi[:], d_wchi[:])
            nc.sync.dma_start(t_womg[:], d_womg[:])

            # =========== chi (site-sharded, all 128 batches) ===========
            # h1 row content is [i(4), ri(2), b(64)]: the stage write from
            # ctanh's [(dn,i) parts, (col,b)] layout is then a 3-dim AP with
            # 128B runs: merged (dn,i) stride 128-els, col stride 7168, b 1.
            q_written = [0] * NQ

            def stage_write(h1re, h1im, npair, pr):
                nonlocal dma_flip
                for u in range(npair):
                    s0 = (pr + u) * CHI_SITES
                    # rectangles (quarter, col0, ncol, dn0, ndn) not crossing
                    # a quarter boundary nor S_LOC
                    pieces = []
                    for colq in range(CHI_COLS):
                        base = s0 + colq * DN
                        ns = min(DN, max(0, S_LOC - base))
                        st = 0
                        while st < ns:
                            s = base + st
                            qh = s // SQ
                            n_ = min(ns - st, (qh + 1) * SQ - s)
                            pieces.append((qh, colq, st, n_))
                            st += n_
                    rects = []
                    ii = 0
                    while ii < len(pieces):
                        qh, colq, st, n_ = pieces[ii]
                        if st == 0 and n_ == DN:
                            jj = ii
                            while (jj + 1 < len(pieces)
                                   and pieces[jj + 1][:1] == (qh,)
                                   and pieces[jj + 1][2] == 0
                                   and pieces[jj + 1][3] == DN
                                   and pieces[jj + 1][1] == pieces[jj][1] + 1):
                                jj += 1
                            rects.append((qh, colq, pieces[jj][1] - colq + 1,
                                          0, DN))
                            ii = jj + 1
                        else:
                            rects.append((qh, colq, 1, st, n_))
                            ii += 1
                    for (qh, c0, ncol, dn0, ndn) in rects:
                        # group index m = local_site*4 + i; multi-col rects
                        # are always full cols (dn0=0, ndn=DN)
                        m0 = (s0 - qh * SQ + c0 * DN + dn0) * 4
                        for ri, tl in ((0, h1re), (1, h1im)):
                            for gg in range(NG):
                                M = d_stq[qh][gg * NP].rearrange(
                                    "s (m x) -> (s m) x", m=4)
                                if ncol == 1:
                                    dst = M[m0:m0 + 4 * ndn,
                                            ri * 64:(ri + 1) * 64]
                                else:
                                    dst = M[m0:m0 + ncol * 4 * DN, :] \
                                        .rearrange("(c m) x -> m c x",
                                                   c=ncol)[
                                        :, :, ri * 64:(ri + 1) * 64]
                                src = tl[u * 64 + dn0 * 4:
                                         u * 64 + (dn0 + ndn) * 4,
                                         c0:c0 + ncol, gg * 64:(gg + 1) * 64]
                                if ncol == 1:
                                    src = src.rearrange("p c b -> p (c b)")
                                nc.sync.dma_start(dst, src)
                        q_written[qh] += ncol * ndn

            dups_done = 0
            dup_rows = 0
            dma_flip = True
            for base in range(0, CHI_NCH, 4):
                nch = min(4, CHI_NCH - base)
                pss = []
                for u in range(nch):
                    ch = base + u
                    gch = pool.tile([128, CHI_COLS, 128], F32, name="gchi",
                                    tag="gchi", bufs=5)
                    nc.gpsimd.dma_gather(
                        gch[:], d_xf[:],
                        t_cgi[:, ch * CHI_COLS * 8:(ch + 1) * CHI_COLS * 8],
                        CHI_COLS * 128, CHI_COLS * 128, 128, elem_step=128)
                    pch = ppsum.tile([128, 512], F32, name="pchi", tag="pchi",
                                     bufs=4)
                    nc.tensor.matmul(
                        pch[:], lhsT=t_wchi[:],
                        rhs=gch[:].rearrange("p a b -> p (a b)"),
                        start=True, stop=True)
                    pss.append(pch)
                h1re = pool.tile([120, 2, CHI_COLS, 128], F16, name="h1re",
                                 tag="h1re", bufs=3)
                h1im = pool.tile([120, 2, CHI_COLS, 128], F16, name="h1im",
                                 tag="h1im", bufs=3)
                emit_ctanh(
                    nc, pool,
                    [(p_[0:56, :], p_[64:120, :], u % 2, u // 2)
                     for u, p_ in enumerate(pss)],
                    h1re[:].rearrange("p g a b -> p (g a b)"),
                    h1im[:].rearrange("p g a b -> p (g a b)"), "ctchi",
                    FG=2)
                for pg in range(2):
                    npair = min(2, nch - 2 * pg)
                    if npair <= 0:
                        break
                    stage_write(h1re[:, pg], h1im[:, pg], npair,
                                base + 2 * pg)
                # fire 512-row dup slices as rows complete; a2a per half
                total_written = sum(q_written)
                while dup_rows < total_written // 512 * 512:
                    r0 = dup_rows
                    qd, lr = r0 // SQ, r0 % SQ
                    for gg in range(NG):
                        for pp_ in range(1, NP):
                            eng = nc.sync if dma_flip else nc.scalar
                            dma_flip = not dma_flip
                            eng.dma_start(
                                d_stq[qd][gg * NP + pp_, lr:lr + 512, :],
                                d_stq[qd][gg * NP, lr:lr + 512, :])
                    dup_rows += 512
                while dups_done < NQ and dup_rows >= (dups_done + 1) * SQ:
                    qd = dups_done
                    nc.gpsimd.collective_compute(
                        "AllToAll", ALU.bypass,
                        replica_groups=[list(range(NCORES))],
                        ins=[d_stq[qd][:]], outs=[d_recv[qd]])
                    dups_done += 1

            # =========== wilson (quarter p, my 64 batches) ===========
            recv_flat = d_recv[:].rearrange("a b c f -> (a b c) f")
            for ch in range(WIL_NCH):
                gw = pool.tile([128, 4, 2, 512], F16, name="gwil", tag="gwil",
                               bufs=4)
                nc.gpsimd.dma_gather(
                    gw[:].rearrange("p a b f -> p (a b) f"), recv_flat,
                    t_wgi[:, ch * 64:(ch + 1) * 64], 4 * WIL_PC, 4 * WIL_PC,
                    512, elem_step=512)
                m1 = pool.tile([128, 2, 512], F16, name="wm1", tag="wm1",
                               bufs=2)
                m2 = pool.tile([128, 2, 512], F16, name="wm2", tag="wm2",
                               bufs=2)
                h2t = pool.tile([128, 2, 512], F16, name="h2t", tag="h2t",
                                bufs=2)
                nc.vector.tensor_mul(m1[:], gw[:, 0, :, :], gw[:, 1, :, :])
                nc.vector.tensor_mul(m2[:], gw[:, 2, :, :], gw[:, 3, :, :])
                # content reorder [i,ri,b] -> [b,ri,i] for omega's
                # K-slices; DVE APs are limited to 3D, so split by (cc, ri)
                for cc in range(2):
                    for ri in range(2):
                        dstv = h2t[:, cc, :].rearrange(
                            "p (b r i) -> r p b i", b=64, r=2, i=4)[ri]
                        nc.vector.scalar_tensor_tensor(
                            out=dstv,
                            in0=m1[:, cc, :].rearrange(
                                "p (i r b) -> r p b i", i=4, r=2, b=64)[ri],
                            scalar=float(WILSON_RESCALE),
                            in1=m2[:, cc, :].rearrange(
                                "p (i r b) -> r p b i", i=4, r=2, b=64)[ri],
                            op0=ALU.mult, op1=ALU.mult)
                jq, r0 = ch // 4, (ch % 4) * WIL_PC
                nc.sync.dma_start(
                    d_h2q[jq][r0:r0 + WIL_PC, :]
                    .rearrange("(c q) f -> q c f", c=2), h2t[:])
                if ch % 4 == 3:
                    nc.gpsimd.collective_compute(
                        "AllGather", ALU.bypass,
                        replica_groups=[[0, 1, 2, 3], [4, 5, 6, 7]],
                        ins=[d_h2q[jq][:]], outs=[d_h2g[jq]])

            # =========== omega (quarter p, my 64 batches) ===========
            h2g_flat = d_h2g[:].rearrange("a b c f -> (a b c) f")
            acc_re = pool.tile([64, 4], F32, name="acc_re")
            acc_im = pool.tile([64, 4], F32, name="acc_im")
            nc.vector.memset(acc_re[:], 0.0)
            nc.vector.memset(acc_im[:], 0.0)

            def omega_tail(zcs, grp):
                # zcs[j]: bh = 2*grp + j//2, half = j%2; pair (pr_, pr_+2)
                # stacks bh 2*grp (rows 0:64) with bh 2*grp+1 (rows 64:128)
                for pr_ in range(2):
                    za, zb = zcs[pr_], zcs[pr_ + 2]
                    tr = pool.tile([128, 512], F32, name="otr", tag="otr",
                                   bufs=2)
                    ti = pool.tile([128, 512], F32, name="oti", tag="oti",
                                   bufs=2)
                    emit_taylor_ctanh(nc, pool, za, zb, tr[:], ti[:])
                    rr = pool.tile([128, 1], F32, name="orr", tag="orr",
                                   bufs=2)
                    rim = pool.tile([128, 1], F32, name="ori", tag="ori",
                                    bufs=2)
                    nc.vector.tensor_reduce(
                        out=rr[:], in_=tr[:], axis=mybir.AxisListType.X,
                        op=ALU.add)
                    nc.vector.tensor_reduce(
                        out=rim[:], in_=ti[:], axis=mybir.AxisListType.X,
                        op=ALU.add)
                    bha, bhb = 2 * grp, 2 * grp + 1
                    nc.vector.tensor_add(
                        acc_re[:, bha:bha + 1], acc_re[:, bha:bha + 1],
                        rr[0:64, :])
                    nc.vector.tensor_add(
                        acc_im[:, bha:bha + 1], acc_im[:, bha:bha + 1],
                        rim[0:64, :])
                    r2 = pool.tile([64, 1], F32, name="or2", tag="or2",
                                   bufs=2)
                    i2 = pool.tile([64, 1], F32, name="oi2", tag="oi2",
                                   bufs=2)
                    nc.vector.tensor_copy(out=r2[:], in_=rr[64:128, :])
                    nc.vector.tensor_copy(out=i2[:], in_=rim[64:128, :])
                    nc.vector.tensor_add(
                        acc_re[:, bhb:bhb + 1], acc_re[:, bhb:bhb + 1],
                        r2[:])
                    nc.vector.tensor_add(
                        acc_im[:, bhb:bhb + 1], acc_im[:, bhb:bhb + 1],
                        i2[:])

            for ch in range(OMG_NCH):
                buf = ch
                for k in range(K_OMG):
                    gk = pool.tile([128, OMG_PC // 128, 512], F16,
                                   name="gomg", tag="gomg", bufs=5)
                    off = (ch * K_OMG + k) * (OMG_PC // 16)
                    nc.gpsimd.dma_gather(
                        gk[:], h2g_flat, t_ogi[:, off:off + OMG_PC // 16],
                        OMG_PC, OMG_PC, 512, elem_step=512)
                    nc.scalar.dma_start(
                        d_gsc[buf, k].rearrange("(c q) f -> q c f",
                                                c=OMG_PC // 128),
                        gk[:])
                for grp in range(2):
                    zcs = [ppsum.tile([128, 512], F32, name=f"zc{jj}",
                                      tag=f"zc{jj}", bufs=1)
                           for jj in range(4)]
                    for k in range(K_OMG):
                        gT = pool.tile([128, 2, OMG_PC], F16, name="gT",
                                       tag="gT", bufs=5)
                        for bi in range(2):
                            bh = 2 * grp + bi
                            nc.sync.dma_start_transpose(
                                gT[:, bi, :],
                                d_gsc[buf, k][:, 128 * bh:128 * (bh + 1)])
                        for bi in range(2):
                            for hh in range(2):
                                nc.tensor.matmul(
                                    zcs[bi * 2 + hh][:],
                                    lhsT=t_womg[:, k * 128:(k + 1) * 128],
                                    rhs=gT[:, bi, 512 * hh:512 * (hh + 1)],
                                    start=(k == 0), stop=(k == K_OMG - 1))
                    omega_tail(zcs, grp)

            if DEBUG_DUMP:
                nc.sync.dma_start(
                    d_recv_d[:],
                    d_recv[:].rearrange("a b c f -> (a b c) f"))
                nc.sync.dma_start(
                    d_h2g_d[:],
                    d_h2g[:].rearrange("a b c f -> (a b c) f"))
            out_t = pool.tile([64, 8], F32, name="out_t")
            nc.vector.tensor_copy(out=out_t[:, 0:4], in_=acc_re[:])
            nc.vector.tensor_copy(out=out_t[:, 4:8], in_=acc_im[:])
            nc.sync.dma_start(d_out[:], out_t[:])
    nc.compile()
    return nc


_NC_CACHE = None


def kernel(x, chi_kernel_idx, chi_kernel_mask, plaquette_idx, plaquette_mask,
           omega_kernel_idx, omega_kernel_mask, chi_w, chi_b, omega_w,
           omega_b, _want_trace=False):
    global _NC_CACHE
    x = np.asarray(x, np.float32)
    chi_kernel_idx = np.asarray(chi_kernel_idx).astype(np.int64)
    plaquette_idx = np.asarray(plaquette_idx).astype(np.int64)
    omega_kernel_idx = np.asarray(omega_kernel_idx).astype(np.int64)
    chi_w = np.asarray(chi_w)
    omega_w = np.asarray(omega_w)

    chi_gidx_cores, wil_gidx_p, omg_gidx_p, wchi, womg = build_host_tables(
        chi_kernel_idx, plaquette_idx, omega_kernel_idx, chi_w, omega_w)

    # omega uses a cubic Taylor tanh; verify |z_omega| is in-range
    _h1 = np.tanh(np.einsum(
        "bnk,ik->bni", x.astype(np.complex64)[:, chi_kernel_idx],
        chi_w[:, 0, :]))
    _gp = _h1[:, plaquette_idx, :]
    _h2 = (WILSON_RESCALE * _gp.real.prod(axis=2)
           + 1j * WILSON_RESCALE * _gp.imag.prod(axis=2))
    _z = np.einsum("bnki,oik->bno", _h2[:, omega_kernel_idx, :], omega_w)
    _zmax = max(np.abs(_z.real).max(), np.abs(_z.imag).max())
    assert _zmax < 0.25, f"omega pre-tanh out of Taylor range: {_zmax}"

    if _NC_CACHE is None:
        _NC_CACHE = build_kernel()
    nc = _NC_CACHE

    xf = np.zeros((N_SITES + 1, 128), np.float32)
    xf[:N_SITES] = x.T
    in_maps = []
    for c in range(NCORES):
        p = c % NP
        in_maps.append({
            "xf": xf, "cgi": chi_gidx_cores[c], "wgi": wil_gidx_p[p],
            "ogi": omg_gidx_p[p], "wchi": wchi, "womg": womg,
        })
    r = run_bass_kernel_spmd(nc, in_maps, core_ids=list(range(NCORES)),
                             trace=_want_trace)
    out = np.zeros(B, np.complex64)
    for c in range(NCORES):
        g, p = c // NP, c % NP
        o = r.results[c]["out"]
        v = (o[:, 0:4] + 1j * o[:, 4:8]).reshape(16, 4, 4)  # [b_lo, o, bh]
        vv = v.sum(axis=1)                                  # [b_lo, bh]
        for bh in range(4):
            out[g * 64 + bh * 16:g * 64 + bh * 16 + 16] += vv[:, bh]
    kernel._LAST_R = r
    if _want_trace:
        kernel._last_result = r
    return out
